# revision 1
# baseline (speedup 1.0000x reference)
"""Trainium2 Bass kernel for nn_Attention_49709951484392 (causal attention
block: LN1 -> QKV -> key smearing -> causal attention with learned ALiBi ->
out-proj -> LN2), sharded over 8 NeuronCores.

Sharding: core c handles batch c//4 and head-group c%4 (4 of 16 heads).
Out-projection partial sums are ReduceScatter'ed over each batch's 4-core
group; each core then runs LN2 on its 512-row slice of the output.

Attention runs in transposed orientation S^T[k, q] so that:
  - the ALiBi term slope*j (j = key position) is a per-partition bias folded
    into the Exp activation,
  - the per-query shift M_i (softmax overflow guard) is folded into the QK
    matmul by augmenting kT with a constant row (8.0) and qT with a row
    holding -M_i (65-dim contraction),
  - the softmax denominator is produced by the PV matmul via an extra ones
    column appended to V (row 64 of oT),
so no transposes of the attention matrix are needed.  M_i is the bound
(|q_i|^2 + max_j|k_j|^2)/16 + relu(slope)*i >= max_j (q_i.k_j/8 + slope*j),
computed with one augmented column-sum matmul per head.
"""
import sys

import numpy as np

sys.path.insert(0, "/opt/trn_rl_repo")

import concourse.bacc as bacc
import concourse.bass as bass
import concourse.mybir as mybir
import concourse.tile as tile
from concourse.bass_utils import run_bass_kernel_spmd
from concourse.masks import make_identity

F32 = mybir.dt.float32
F32R = mybir.dt.float32r
BF16 = mybir.dt.bfloat16
AF = mybir.ActivationFunctionType
ALU = mybir.AluOpType
AX = mybir.AxisListType

HEADS = 16
DH = 64
DM = 1024
B, L = 2, 2048
EPS = 1e-5
NCORES = 8
HG = 4          # heads per core
FL = HG * DH    # local feature width (256)
QB = 1024       # query block
NK = L // 128   # key blocks of 128
NLT = L // 128  # l-tiles

_CACHE = {}
PHASE_MARKS = []


def _mark(name, nc):
    ids = []
    for k in nc.inst_map.keys():
        if isinstance(k, str) and k.startswith("I-"):
            try:
                ids.append(int(k.split("-")[1]))
            except ValueError:
                pass
    PHASE_MARKS.append((name, max(ids) if ids else 0))


import os
PHASE_LIMIT = int(os.environ.get("KPHASES", "9"))


def _build_program():
    nc = bacc.Bacc()
    xin = nc.declare_dram_parameter("xin", [L, DM], F32, isOutput=False)
    wqk_d = nc.declare_dram_parameter("wqk", [DM, 2 * FL], F32R, isOutput=False)
    wv_d = nc.declare_dram_parameter("wv", [DM, FL], F32R, isOutput=False)
    wo_d = nc.declare_dram_parameter("wo", [FL, DM], F32R, isOutput=False)
    bqk_d = nc.declare_dram_parameter("bqk", [2 * FL, 1], F32, isOutput=False)
    bv_d = nc.declare_dram_parameter("bv", [HG * 65], F32, isOutput=False)
    bqkr_d = nc.declare_dram_parameter("bqkr", [1, 2 * FL], F32R, isOutput=False)
    bvr_d = nc.declare_dram_parameter("bvr", [1, FL], F32R, isOutput=False)
    srep_d = nc.declare_dram_parameter("srep", [FL, 1], F32, isOutput=False)
    omsrep_d = nc.declare_dram_parameter("omsrep", [FL, 1], F32, isOutput=False)
    alibi_d = nc.declare_dram_parameter("alibi", [HG, 128, NK], F32, isOutput=False)
    aliq_d = nc.declare_dram_parameter("aliq", [HG, L], F32R, isOutput=False)
    ln2g_d = nc.declare_dram_parameter("ln2g", [DM], F32, isOutput=False)
    ln2b_d = nc.declare_dram_parameter("ln2b", [DM], F32, isOutput=False)
    out_d = nc.declare_dram_parameter("out", [L // 4, DM], F32, isOutput=True)

    from contextlib import ExitStack
    with tile.TileContext(nc) as tc, ExitStack() as ctx:
        _emit(ctx, nc, tc, xin, wqk_d, wv_d, wo_d, bqk_d, bv_d, bqkr_d, bvr_d,
              srep_d, omsrep_d, alibi_d, aliq_d, ln2g_d, ln2b_d, out_d)
    nc.compile()
    return nc


def _bcast_ap(handle, parts, free):
    ap = handle[:]
    return bass.AP(tensor=ap.tensor, offset=0, ap=[[0, parts], [1, free]])


def _emit(ctx, nc, tc, xin, wqk_d, wv_d, wo_d, bqk_d, bv_d, bqkr_d, bvr_d,
          srep_d, omsrep_d, alibi_d, aliq_d, ln2g_d, ln2b_d, out_d):
    from contextlib import ExitStack

    consts = ctx.enter_context(tc.tile_pool(name="consts", bufs=1))
    persist = ctx.enter_context(tc.tile_pool(name="persist", bufs=1))
    dram = ctx.enter_context(tc.tile_pool(name="dram", bufs=1, space="DRAM"))

    ident = consts.tile([128, 128], F32)
    make_identity(nc, ident)
    eps_t = consts.tile([128, 1], F32)
    nc.vector.memset(eps_t, EPS)
    ones64_f = consts.tile([1, 64], F32)
    nc.vector.memset(ones64_f, 1.0)
    ones64_r = consts.tile([1, 64], F32R)
    nc.vector.tensor_copy(out=ones64_r, in_=ones64_f)
    onescol_f = consts.tile([64, 1], F32)
    nc.vector.memset(onescol_f, 1.0)
    onescol_r = consts.tile([64, 1], F32R)
    nc.vector.tensor_copy(out=onescol_r, in_=onescol_f)
    onesvcol_f = consts.tile([128, HG], F32)
    nc.vector.memset(onesvcol_f, 1.0)
    bd_f = consts.tile([128, 2], F32)
    nc.vector.memset(bd_f, 0.0)
    nc.vector.memset(bd_f[0:64, 0:1], 1.0)
    nc.vector.memset(bd_f[64:128, 1:2], 1.0)
    bd_r = consts.tile([128, 2], F32R)
    nc.vector.tensor_copy(out=bd_r, in_=bd_f)
    ones512_f = consts.tile([1, 512], F32)
    nc.vector.memset(ones512_f, 1.0)
    ones512_r = consts.tile([1, 512], F32R)
    nc.vector.tensor_copy(out=ones512_r, in_=ones512_f)
    bqkr_t = consts.tile([1, 2 * FL], F32R)
    nc.scalar.dma_start(out=bqkr_t, in_=bqkr_d[:, :])
    bvr_t = consts.tile([1, FL], F32R)
    nc.scalar.dma_start(out=bvr_t, in_=bvr_d[:, :])
    # mask[p, f] = 1 where p > f (the causally-invalid part of a diag block)
    trimask = consts.tile([128, 128], mybir.dt.int8)
    nc.gpsimd.memset(trimask, 1)
    nc.gpsimd.affine_select(out=trimask, in_=trimask, compare_op=ALU.is_ge,
                            fill=0, base=-1, channel_multiplier=1,
                            pattern=[[-1, 128]])
    zeros_r = consts.tile([128, 128], F32R)
    zeros_f = consts.tile([128, 128], F32)
    nc.vector.memset(zeros_f, 0.0)
    nc.vector.tensor_copy(out=zeros_r, in_=zeros_f)

    # small runtime vectors (scalar-engine HWDGE; keep SP free for x and
    # the pool queue free for the first LN applies)
    bqk_t = [consts.tile([128, 1], F32, name=f"bqk{m}") for m in range(4)]
    for m in range(4):
        nc.scalar.dma_start(out=bqk_t[m], in_=bqk_d[m * 128:(m + 1) * 128, :])
    oms_t = [consts.tile([128, 1], F32, name=f"oms{m}") for m in range(2)]
    for m in range(2):
        nc.scalar.dma_start(out=oms_t[m], in_=omsrep_d[m * 128:(m + 1) * 128, :])
    s_t = [consts.tile([128, 1], F32, name=f"sr{m}") for m in range(2)]
    for m in range(2):
        nc.scalar.dma_start(out=s_t[m], in_=srep_d[m * 128:(m + 1) * 128, :])
    alibi_t = [consts.tile([128, NK], F32, name=f"ali{h}") for h in range(HG)]
    for h in range(HG):
        nc.gpsimd.dma_start(out=alibi_t[h], in_=alibi_d[h, :, :])

    # persistent activation tiles (qT/kT rows 0:64 = head data, row 64 = aug)
    qT = [persist.tile([65, L], F32R, name=f"qT{h}") for h in range(HG)]
    kT = [persist.tile([65, L], F32R, name=f"kT{h}") for h in range(HG)]

    # ---- Phases 1..2b: need hT resident ----
    with ExitStack() as s1:
        hTp = s1.enter_context(tc.tile_pool(name="hTp", bufs=1))
        hT = [hTp.tile([128, 4, L], F32R, name=f"hT{g}") for g in range(2)]
        s1w = s1.enter_context(ExitStack())
        wp = s1w.enter_context(tc.tile_pool(name="wp", bufs=1))
        wqk8 = wp.tile([128, 8, 2 * FL], F32R, name="wqk8")
        wqk_t = [wqk8[:, kc, :] for kc in range(8)]

        sqp = s1.enter_context(tc.tile_pool(name="sqp", bufs=1))
        qn_bf = [sqp.tile([2, L], BF16, name=f"qn{p}") for p in range(2)]
        kmx = [sqp.tile([2, 4], F32, name=f"kmx{p}") for p in range(2)]
        wvp = ctx.enter_context(tc.tile_pool(name="wvp", bufs=1, side="right"))
        wv8 = wvp.tile([128, 8, FL], F32R, name="wv8")
        wv_t = [wv8[:, kc, :] for kc in range(8)]

        _mark('ph1', nc)
        # Phases 1+2a fused: per group of 4 l-tiles, LN1+transpose then the
        # QK GEMM N-tile over those columns — keeps PE streaming.
        with ExitStack() as ph1:
            xp = ph1.enter_context(tc.tile_pool(name="xp", bufs=2))
            x4p = ph1.enter_context(tc.tile_pool(name="x4p", bufs=2))
            stp = ph1.enter_context(tc.tile_pool(name="stp", bufs=6))
            psT = ph1.enter_context(tc.tile_pool(name="psT", bufs=3, space="PSUM"))
            psq = ph1.enter_context(tc.tile_pool(name="psq", bufs=2, space="PSUM"))
            ktp = ph1.enter_context(tc.tile_pool(name="ktp", bufs=1))
            psn2 = ph1.enter_context(tc.tile_pool(name="psn2", bufs=2,
                                                  space="PSUM"))
            xr = xin.rearrange("(i j p) d -> i p j d", j=2, p=128)
            x4 = None
            kbcol = {}
            for n in range(4):
                for j4 in range(4):
                    lt = 4 * n + j4
                    if lt % 2 == 0:
                        x4 = x4p.tile([128, 2, DM], F32, name="x4", tag="x4")
                        nc.sync.dma_start(out=x4, in_=xr[lt // 2])
                        if lt == 2:
                            # weights after the first two x groups are queued
                            nc.sync.dma_start(
                                out=wqk8,
                                in_=wqk_d.rearrange("(c p) n -> p c n", p=128))
                            nc.sync.dma_start(
                                out=wv8,
                                in_=wv_d.rearrange("(c p) n -> p c n", p=128))
                    x_t = x4[:, lt % 2, :]
                    st = stp.tile([128, 2, 6], F32)
                    nc.vector.bn_stats(out=st[:, 0, :], in_=x_t[:, 0:512])
                    nc.vector.bn_stats(out=st[:, 1, :], in_=x_t[:, 512:1024])
                    mv = stp.tile([128, 2], F32)
                    nc.vector.bn_aggr(out=mv, in_=st)
                    rstd = stp.tile([128, 1], F32)
                    nc.scalar.activation(out=rstd, in_=mv[:, 1:2], func=AF.Sqrt,
                                         bias=eps_t, scale=1.0)
                    nc.vector.reciprocal(out=rstd, in_=rstd)
                    h_t = xp.tile([128, DM], F32)
                    eng = nc.vector if lt == 0 else nc.gpsimd
                    eng.tensor_scalar(out=h_t, in0=x_t, scalar1=mv[:, 0:1],
                                      scalar2=rstd, op0=ALU.subtract,
                                      op1=ALU.mult)
                    for g in range(2):
                        pst = psT.tile([128, 512], F32)
                        for j in range(4):
                            dc = 4 * g + j
                            nc.tensor.transpose(pst[:, j * 128:(j + 1) * 128],
                                                h_t[:, dc * 128:(dc + 1) * 128],
                                                ident)
                        ceng = nc.scalar.copy if g == 0 else \
                            (lambda out, in_: nc.vector.tensor_copy(out=out,
                                                                    in_=in_))
                        ceng(out=hT[g][:, :, lt * 128:(lt + 1) * 128],
                             in_=pst.rearrange("p (a b) -> p a b", a=4))
                # QK GEMM for this N-tile (columns 4n*128 .. 4n*128+512)
                nsl = slice(n * 512, (n + 1) * 512)
                for m in range(4):      # 0,1: q head-pairs; 2,3: k head-pairs
                    pair = m % 2
                    is_q = m < 2
                    ps = psq.tile([128, 512], F32, name="psqk", tag="psqk")
                    for kc in range(8):
                        nc.tensor.matmul(
                            ps, wqk_t[kc][:, m * 128:(m + 1) * 128],
                            hT[kc // 4][:, kc % 4, nsl],
                            start=(kc == 0), stop=False)
                    nc.tensor.matmul(ps, bqkr_t[:, m * 128:(m + 1) * 128],
                                     ones512_r, start=False, stop=True)
                    # row-norm statistics: sq = (x + b)^2 on ACT, then a
                    # blockdiag column-sum -> per-head-pair norms
                    sq_t = sqp.tile([128, 512], F32R, name="sq", tag="sq",
                                    bufs=2)
                    nc.scalar.activation(out=sq_t, in_=ps, func=AF.Square,
                                         bias=0.0, scale=1.0)
                    pn2 = psn2.tile([2, 512], F32, name="pn2", tag="pn2")
                    nc.tensor.matmul(pn2, bd_r, sq_t, start=True, stop=True)
                    if is_q:
                        nc.scalar.copy(out=qn_bf[pair][:, nsl], in_=pn2)
                    else:
                        nc.vector.reduce_max(out=kmx[pair][:, n:n + 1],
                                             in_=pn2, axis=AX.X)
                    for hh in range(2):
                        h = pair * 2 + hh
                        rows = slice(hh * 64, (hh + 1) * 64)
                        if is_q:
                            nc.scalar.copy(out=qT[h][0:64, nsl],
                                           in_=ps[rows, :])
                            continue
                        # k already biased: kT = k*(1-s); tmp = k*s; the
                        # shifted add completes the smear per column block
                        nc.vector.tensor_scalar(
                            out=kT[h][0:64, nsl], in0=ps[rows, :],
                            scalar1=oms_t[pair][rows, :], scalar2=None,
                            op0=ALU.mult)
                        tmp = ktp.tile([64, 512], F32, name="ktmp",
                                       tag="ktmp", bufs=3)
                        nc.vector.tensor_scalar(
                            out=tmp, in0=ps[rows, :],
                            scalar1=s_t[pair][rows, :], scalar2=None,
                            op0=ALU.mult)
                        c0 = n * 512
                        nc.gpsimd.tensor_tensor(
                            out=kT[h][0:64, c0 + 1:c0 + 512],
                            in0=kT[h][0:64, c0 + 1:c0 + 512],
                            in1=tmp[:, 0:511], op=ALU.add)
                        if n > 0:
                            nc.gpsimd.tensor_tensor(
                                out=kT[h][0:64, c0:c0 + 1],
                                in0=kT[h][0:64, c0:c0 + 1],
                                in1=kbcol[h][:, 0:1], op=ALU.add)
                        if n < 3:
                            bc = ktp.tile([64, 1], F32, name=f"kb{h}",
                                          tag=f"kb{h}", bufs=2)
                            nc.gpsimd.tensor_copy(out=bc, in_=tmp[:, 511:512])
                            kbcol[h] = bc

        _mark('ph2a', nc)
        # Phase 2a: kT row 64 = 8.0 (the augmentation constant)
        with ExitStack() as ph2:
            ktp2 = ph2.enter_context(tc.tile_pool(name="ktp2", bufs=1))
            const8_f = ktp2.tile([1, 512], F32, name="const8")
            nc.vector.memset(const8_f, 8.0)
            for h in range(HG):
                for n8 in range(4):
                    nc.vector.tensor_copy(
                        out=kT[h][64:65, n8 * 512:(n8 + 1) * 512], in_=const8_f)

        _mark('ph3', nc)
        # ---- Phase 3: -M rows of qT from the inline norms:
        #      -M = -(qn + kmax^2)/16 - relu(slope)*i, scattered per head ----
        if PHASE_LIMIT < 3:
            return
        with ExitStack() as s2:
            mtp = s2.enter_context(tc.tile_pool(name="mtp", bufs=2))
            for pair in range(2):
                kms2 = mtp.tile([2, 1], F32, name="kms2", tag="kms2")
                nc.vector.reduce_max(out=kms2, in_=kmx[pair], axis=AX.X)
                aliq2 = mtp.tile([2, L], F32R, name="aliq2", tag="aliq2")
                nc.sync.dma_start(out=aliq2,
                                  in_=aliq_d[pair * 2:pair * 2 + 2, :])
                stag = mtp.tile([2, L], F32R, name="stag", tag="stag")
                with nc.allow_low_precision(reason="f32r is f32 bits"):
                    nc.vector.tensor_scalar(out=stag, in0=qn_bf[pair],
                                            scalar1=kms2, scalar2=-1.0 / 16.0,
                                            op0=ALU.add, op1=ALU.mult)
                nc.gpsimd.tensor_tensor(out=stag, in0=stag, in1=aliq2,
                                        op=ALU.subtract)
                for hh in range(2):
                    nc.sync.dma_start(out=qT[pair * 2 + hh][64:65, :],
                                      in_=stag[hh:hh + 1, :])

        _mark('ph2b', nc)
        # Phase 2b setup: v pools on the right side; first half (l-tiles 0..7)
        # runs before attention, second half is emitted after q-chunk 0.
        vp = ctx.enter_context(tc.tile_pool(name="vp", bufs=1, side="right"))
        v_sb = vp.tile([128, NLT, HG, 65], F32R)
        psv = s1.enter_context(tc.tile_pool(name="psv", bufs=2, space="PSUM"))

        ones128_f = vp.tile([1, 128], F32)
        nc.vector.memset(ones128_f, 1.0)
        ones128_r = vp.tile([1, 128], F32R)
        nc.vector.tensor_copy(out=ones128_r, in_=ones128_f)

        def emit_v(lt_range):
            for lt in lt_range:
                ps = psv.tile([128, FL], F32, name="psv", tag="psv")
                for kc in range(8):
                    nc.tensor.matmul(
                        ps, hT[kc // 4][:, kc % 4, lt * 128:(lt + 1) * 128],
                        wv_t[kc], start=(kc == 0), stop=False)
                nc.tensor.matmul(ps, ones128_r, bvr_t, start=False, stop=True)
                nc.scalar.copy(
                    out=v_sb[:, lt, :, 0:64],
                    in_=ps.rearrange("p (a b) -> p a b", a=HG))
                nc.vector.tensor_copy(
                    out=v_sb[:, lt, :, 64:65],
                    in_=onesvcol_f.rearrange("p (a b) -> p a b", a=HG))

        emit_v(range(8))

        emit_v(range(8, NLT))

    # ---- Phases 4..5 interleaved: per q-chunk: attention (all heads),
    #      out-proj, chunked ReduceScatter, LN2 — RS hides under compute ----
    if PHASE_LIMIT < 4:
        return
    NCH = L // QB  # chunks (2)
    with ExitStack() as s3:
        oTp = s3.enter_context(tc.tile_pool(name="oTp", bufs=1))
        oT = [oTp.tile([128, L], F32R, name=f"oT{m}") for m in range(2)]
        psS = s3.enter_context(tc.tile_pool(name="psS", bufs=2, space="PSUM"))
        psO = s3.enter_context(tc.tile_pool(name="psO", bufs=1, space="PSUM"))
        psY = s3.enter_context(tc.tile_pool(name="psY", bufs=2, space="PSUM"))
        atp = s3.enter_context(tc.tile_pool(name="atp", bufs=4))
        nrm = s3.enter_context(tc.tile_pool(name="nrm", bufs=3))
        wop = s3.enter_context(tc.tile_pool(name="wop", bufs=1))
        ysp = s3.enter_context(tc.tile_pool(name="ysp", bufs=3))
        wo2 = wop.tile([128, 2, DM], F32R, name="wo2")
        nc.sync.dma_start(out=wo2, in_=wo_d.rearrange("(c p) n -> p c n", p=128))
        wo_t = [wo2[:, kc, :] for kc in range(2)]
        g2b_t = wop.tile([128, DM], F32)
        nc.gpsimd.dma_start(out=g2b_t, in_=_bcast_ap(ln2g_d, 128, DM))
        b2b_t = wop.tile([128, DM], F32)
        nc.gpsimd.dma_start(out=b2b_t, in_=_bcast_ap(ln2b_d, 128, DM))
        ypart = [dram.tile([QB, DM], BF16, name=f"ypart{i}") for i in range(2)]
        yred = [dram.tile([QB // 4, DM], BF16, name=f"yred{i}") for i in range(2)]
        do_proj = PHASE_LIMIT >= 5

        for qb in range(NCH):
            qlo = qb * QB
            for h in range(HG):
                ops = psO.tile([65, QB], F32, name="ops", tag="ops")
                nkb = (qlo + QB) // 128
                # last k-block index that writes each 512-wide psum bank
                last_kbi = [(qlo + 512) // 128 - 1, nkb - 1]
                for kbi in range(nkb):
                    kb = kbi * 128
                    off = max(0, kb - qlo)
                    sps = psS.tile([128, QB], F32, name="sps", tag="sps")
                    for half in range(2):
                        r0, r1 = max(off, half * 512), (half + 1) * 512
                        if r0 >= r1:
                            continue
                        nc.tensor.matmul(sps[:, r0:r1], kT[h][:, kb:kb + 128],
                                         qT[h][:, qlo + r0:qlo + r1],
                                         start=True, stop=True)
                    at = atp.tile([128, QB], F32R, name="at", tag="at")
                    nc.scalar.activation(out=at[:, off:QB],
                                         in_=sps[:, off:QB], func=AF.Exp,
                                         bias=alibi_t[h][:, kbi:kbi + 1],
                                         scale=0.125)
                    if kb >= qlo:
                        nc.gpsimd.affine_select(
                            out=at[:, off:off + 128],
                            in_=at[:, off:off + 128],
                            compare_op=ALU.is_ge, fill=0.0, base=0,
                            channel_multiplier=-1, pattern=[[1, 128]])
                    for half in range(2):
                        r0, r1 = max(off, half * 512), (half + 1) * 512
                        if r0 >= r1:
                            continue
                        nc.tensor.matmul(ops[:, r0:r1], v_sb[:, kbi, h, :],
                                         at[:, r0:r1],
                                         start=(kbi == 0),
                                         stop=(kbi == last_kbi[half]))
                # normalize rows 0:64 by 1/denom (row 64), store into oT
                dr_r = nrm.tile([1, QB], F32R, name="drr", tag="drr")
                with nc.allow_low_precision(reason="f32r is f32 bits"):
                    nc.vector.reciprocal(out=dr_r, in_=ops[64:65, :])
                bps = psS.tile([64, QB], F32, name="bps", tag="sps")
                for half in range(2):
                    nc.tensor.matmul(bps[:, half * 512:(half + 1) * 512],
                                     ones64_r,
                                     dr_r[:, half * 512:(half + 1) * 512],
                                     start=True, stop=True)
                bsb = nrm.tile([64, QB], F32, name="bsb", tag="bsb")
                nc.vector.tensor_copy(out=bsb, in_=bps)
                if h % 2 == 0:
                    nc.vector.tensor_mul(out=oT[h // 2][0:64, qlo:qlo + QB],
                                         in0=ops[0:64, :], in1=bsb)
                else:
                    ostg = nrm.tile([64, QB], F32R, name="ostg", tag="ostg")
                    nc.vector.tensor_mul(out=ostg, in0=ops[0:64, :], in1=bsb)
                    nc.sync.dma_start(out=oT[h // 2][64:128, qlo:qlo + QB],
                                       in_=ostg)

            # out-proj for this q-chunk + its bf16 ReduceScatter
            if not do_proj:
                continue
            for j in range(QB // 128):
                lt = qb * (QB // 128) + j
                ysb = ysp.tile([128, DM], BF16, name="ysb", tag="ysb")
                for n in range(2):
                    ps = psY.tile([128, 512], F32, name="psy", tag="psy")
                    for kc in range(2):
                        nc.tensor.matmul(ps,
                                         oT[kc][:, lt * 128:(lt + 1) * 128],
                                         wo_t[kc][:, n * 512:(n + 1) * 512],
                                         start=(kc == 0), stop=(kc == 1))
                    nc.vector.tensor_copy(out=ysb[:, n * 512:(n + 1) * 512],
                                          in_=ps)
                nc.sync.dma_start(out=ypart[qb][j * 128:(j + 1) * 128, :],
                                  in_=ysb)
            if PHASE_LIMIT >= 6:
                nc.gpsimd.collective_compute(
                    "ReduceScatter", ALU.add,
                    replica_groups=[[0, 1, 2, 3], [4, 5, 6, 7]],
                    ins=[ypart[qb][:, :]], outs=[yred[qb][:, :]])

        # LN2 per chunk (overlaps with later chunks' compute via deps)
        if PHASE_LIMIT < 7:
            return
        for sc in range(4):
            y_t = ysp.tile([128, DM], BF16, name="y2t", tag="y2t")
            nc.sync.dma_start(out=y_t, in_=yred[sc // 2][(sc % 2) * 128:
                                                         (sc % 2 + 1) * 128, :])
            st = ysp.tile([128, 2, 6], F32, name="st2", tag="st2")
            nc.vector.bn_stats(out=st[:, 0, :], in_=y_t[:, 0:512])
            nc.vector.bn_stats(out=st[:, 1, :], in_=y_t[:, 512:1024])
            mv = ysp.tile([128, 2], F32, name="mv2", tag="mv2")
            nc.vector.bn_aggr(out=mv, in_=st)
            rstd = ysp.tile([128, 1], F32, name="rstd2", tag="rstd2")
            nc.scalar.activation(out=rstd, in_=mv[:, 1:2], func=AF.Sqrt,
                                 bias=eps_t, scale=1.0)
            nc.vector.reciprocal(out=rstd, in_=rstd)
            o_t = ysp.tile([128, DM], F32, name="o2t", tag="o2t")
            nc.gpsimd.tensor_scalar(out=o_t, in0=y_t, scalar1=mv[:, 0:1],
                                    scalar2=rstd, op0=ALU.subtract,
                                    op1=ALU.mult)
            nc.gpsimd.tensor_tensor(out=o_t, in0=o_t, in1=g2b_t, op=ALU.mult)
            nc.gpsimd.tensor_tensor(out=o_t, in0=o_t, in1=b2b_t, op=ALU.add)
            nc.sync.dma_start(out=out_d[sc * 128:(sc + 1) * 128, :], in_=o_t)


def _prep_inputs(x, ln1_g, ln1_b, in_w, out_w, ln2_g, ln2_b, slopes, smear):
    """Slice/transpose per-core views of the weights (host-side marshaling)."""
    x = np.asarray(x, np.float32)
    in_w = np.asarray(in_w, np.float32)
    out_w = np.asarray(out_w, np.float32)
    ln1_g = np.asarray(ln1_g, np.float32)
    ln1_b = np.asarray(ln1_b, np.float32)
    slopes = np.asarray(slopes, np.float32)
    smear = np.asarray(smear, np.float32)
    w_eff = in_w * ln1_g[None, :]
    qkvb = in_w @ ln1_b
    sig = 1.0 / (1.0 + np.exp(-smear))
    in_maps = []
    for c in range(NCORES):
        b, hg = c // 4, c % 4
        f0 = FL * hg
        wq = w_eff[f0:f0 + FL]
        wk = w_eff[DM + f0:DM + f0 + FL]
        wv = w_eff[2 * DM + f0:2 * DM + f0 + FL]
        sl4 = slopes[4 * hg:4 * hg + 4]
        p = np.arange(128, dtype=np.float32)
        kbv = np.arange(NK, dtype=np.float32) * 128.0
        alibi = sl4[:, None, None] * (kbv[None, None, :] + p[None, :, None])
        aliq = np.maximum(sl4, 0.0)[:, None] * np.arange(L, dtype=np.float32)[None, :]
        bvp = np.zeros((HG, 65), np.float32)
        bvp[:, :64] = qkvb[2 * DM + f0:2 * DM + f0 + FL].reshape(HG, 64)
        in_maps.append({
            "xin": np.ascontiguousarray(x[b]),
            "wqk": np.ascontiguousarray(np.concatenate([wq, wk], 0).T),
            "wv": np.ascontiguousarray(wv.T),
            "wo": np.ascontiguousarray(out_w[:, f0:f0 + FL].T),
            "bqk": np.ascontiguousarray(
                np.concatenate([qkvb[f0:f0 + FL],
                                qkvb[DM + f0:DM + f0 + FL]])[:, None]),
            "bv": bvp.reshape(-1),
            "bqkr": np.ascontiguousarray(
                np.concatenate([qkvb[f0:f0 + FL],
                                qkvb[DM + f0:DM + f0 + FL]])[None, :]),
            "bvr": np.ascontiguousarray(
                qkvb[2 * DM + f0:2 * DM + f0 + FL][None, :]),
            "srep": np.repeat(sig[4 * hg:4 * hg + 4], 64)[:, None].astype(np.float32),
            "omsrep": np.repeat(1.0 - sig[4 * hg:4 * hg + 4], 64)[:, None].astype(np.float32),
            "alibi": np.ascontiguousarray(alibi.astype(np.float32)),
            "aliq": np.ascontiguousarray(aliq.astype(np.float32)),
            "ln2g": np.asarray(ln2_g, np.float32),
            "ln2b": np.asarray(ln2_b, np.float32),
        })
    return in_maps


def kernel(**inputs):
    if "nc" not in _CACHE:
        _CACHE["nc"] = _build_program()
    nc = _CACHE["nc"]
    in_maps = _prep_inputs(**inputs)
    res = run_bass_kernel_spmd(nc, in_maps, core_ids=list(range(NCORES)))
    out = np.empty((B, L, DM), np.float32)
    for c in range(NCORES):
        b, hg = c // 4, c % 4
        r = res.results[c]["out"]
        for sc in range(4):
            g0 = (sc // 2) * 1024 + 256 * hg + (sc % 2) * 128
            out[b, g0:g0 + 128, :] = r[sc * 128:(sc + 1) * 128, :]
    return out



# revision 59
# speedup vs baseline: 1.0646x; 1.0646x over previous
"""Trainium2 Bass kernel for nn_Attention_49709951484392 (causal attention
block: LN1 -> QKV -> key smearing -> causal attention with learned ALiBi ->
out-proj -> LN2), sharded over 8 NeuronCores.

Sharding: core c handles batch c//4 and head-group c%4 (4 of 16 heads).
Out-projection partial sums are ReduceScatter'ed over each batch's 4-core
group; each core then runs LN2 on its 512-row slice of the output.

Attention runs in transposed orientation S^T[k, q] so that:
  - the ALiBi term slope*j (j = key position) is a per-partition bias folded
    into the Exp activation,
  - the per-query shift M_i (softmax overflow guard) is folded into the QK
    matmul by augmenting kT with a constant row (8.0) and qT with a row
    holding -M_i (65-dim contraction),
  - the softmax denominator is produced by the PV matmul via an extra ones
    column appended to V (row 64 of the PV output),
so no transposes of the attention matrix are needed.  M_i is the bound
(|q_i|^2 + max_j|k_j|^2)/16 + relu(slope)*i >= max_j (q_i.k_j/8 + slope*j),
computed with one augmented column-sum matmul per head; kmax is taken over
only the key n-tiles a query chunk can see, so chunk n's attention starts
right after GEMM n-tile n.

The emission is one fused loop over the four 512-column n-tiles:
  LN1+transpose group n -> QK GEMM n -> V GEMM group n -> -M rows chunk n ->
  attention chunk n (out-proj of chunk n-1 interleaved per head) ->
  ReduceScatter chunk n-1 -> LN2 chunk n-2
which keeps the in-order PE stream dense and hides the collectives.

Activations/weights are bf16 on the PE (f32 PSUM accumulation); the act
table dict is reordered so Exp/Ln/Copy/Square all live in one function set
(no LoadActFuncSet churn).
"""
import sys

import numpy as np
import ml_dtypes

sys.path.insert(0, "/opt/trn_rl_repo")

import concourse.bacc as bacc
import concourse.bass as bass
import concourse.mybir as mybir
import concourse.tile as tile
from concourse.bass_utils import run_bass_kernel_spmd
from concourse.masks import make_identity

F32 = mybir.dt.float32
F32R = mybir.dt.float32r
BF16 = mybir.dt.bfloat16
AF = mybir.ActivationFunctionType
ALU = mybir.AluOpType
AX = mybir.AxisListType

HEADS = 16
DH = 64
DM = 1024
B, L = 2, 2048
EPS = 1e-5
NCORES = 8
HG = 4          # heads per core
FL = HG * DH    # local feature width (256)
QB = 512        # query chunk == n-tile width
NK = L // 128   # key blocks of 128
NLT = L // 128  # l-tiles
NCH = L // QB   # chunks (4)

_CACHE = {}
PHASE_MARKS = []


def _mark(name, nc):
    ids = []
    for k in nc.inst_map.keys():
        if isinstance(k, str) and k.startswith("I-"):
            try:
                ids.append(int(k.split("-")[1]))
            except ValueError:
                pass
    PHASE_MARKS.append((name, max(ids) if ids else 0))


def _patch_act_tables():
    """Put the set containing both Exp and Ln first so the act-table pass
    assigns every activation in this kernel to one set (zero reloads)."""
    import concourse.hw_specs as hws
    if getattr(bacc, "_act_tables_patched", False):
        return
    orig = hws.get_activation_tables

    def constrained(module_arch):
        # Keep canonical set order/ids (the runtime keys tables by id), but
        # hide Exp/Ln from every set except the one containing both, so the
        # table-load pass assigns all our activations to that single set.
        t = orig(module_arch)
        import concourse.mybir as mb
        AFt = mb.ActivationFunctionType
        want = {AFt.Exp, AFt.Ln, AFt.Copy, AFt.Square}
        best = None
        for name, funcs in t.items():
            if want <= funcs:
                best = name
                break
        if best is None:
            return t
        out = {}
        for name, funcs in t.items():
            if name == best:
                out[name] = funcs
            else:
                out[name] = funcs - {AFt.Exp, AFt.Ln}
        return out

    bacc.get_activation_tables = constrained
    bacc._act_tables_patched = True


def _build_program():
    _patch_act_tables()
    nc = bacc.Bacc()
    xin = nc.declare_dram_parameter("xin", [L, DM], F32, isOutput=False)
    wqk_d = nc.declare_dram_parameter("wqk", [DM, 2 * FL], BF16, isOutput=False)
    wv_d = nc.declare_dram_parameter("wv", [DM, FL], BF16, isOutput=False)
    wo_d = nc.declare_dram_parameter("wo", [FL, DM], BF16, isOutput=False)
    bqkr_d = nc.declare_dram_parameter("bqkr", [1, 2 * FL], BF16, isOutput=False)
    bvr_d = nc.declare_dram_parameter("bvr", [1, FL], BF16, isOutput=False)
    srep_d = nc.declare_dram_parameter("srep", [FL, 1], F32, isOutput=False)
    omsrep_d = nc.declare_dram_parameter("omsrep", [FL, 1], F32, isOutput=False)
    alibi_d = nc.declare_dram_parameter("alibi", [HG, 128, NK], F32, isOutput=False)
    aliq_d = nc.declare_dram_parameter("aliq", [HG, L], BF16, isOutput=False)
    ln2g_d = nc.declare_dram_parameter("ln2g", [DM], F32, isOutput=False)
    ln2b_d = nc.declare_dram_parameter("ln2b", [DM], F32, isOutput=False)
    out_d = nc.declare_dram_parameter("out", [L // 4, DM], F32, isOutput=True)

    from contextlib import ExitStack
    with tile.TileContext(nc) as tc, ExitStack() as ctx:
        _emit(ctx, nc, tc, xin, wqk_d, wv_d, wo_d, bqkr_d, bvr_d,
              srep_d, omsrep_d, alibi_d, aliq_d, ln2g_d, ln2b_d, out_d)
    nc.compile()
    return nc


def _bcast_ap(handle, parts, free):
    ap = handle[:]
    return bass.AP(tensor=ap.tensor, offset=0, ap=[[0, parts], [1, free]])


def _emit(ctx, nc, tc, xin, wqk_d, wv_d, wo_d, bqkr_d, bvr_d,
          srep_d, omsrep_d, alibi_d, aliq_d, ln2g_d, ln2b_d, out_d):
    consts = ctx.enter_context(tc.tile_pool(name="consts", bufs=1))
    persist = ctx.enter_context(tc.tile_pool(name="persist", bufs=1))
    dram = ctx.enter_context(tc.tile_pool(name="dram", bufs=1, space="DRAM"))

    ident = consts.tile([128, 128], F32)
    make_identity(nc, ident)
    identb = consts.tile([128, 128], BF16)
    nc.vector.tensor_copy(out=identb, in_=ident)
    eps_t = consts.tile([128, 1], F32)
    nc.vector.memset(eps_t, EPS)
    ones64_f = consts.tile([1, 64], F32)
    nc.vector.memset(ones64_f, 1.0)
    ones64_r = consts.tile([1, 64], F32R)
    nc.vector.tensor_copy(out=ones64_r, in_=ones64_f)
    onesvcol_b = consts.tile([128, HG], BF16)
    nc.vector.memset(onesvcol_b, 1.0)
    bd_f = consts.tile([128, 2], F32)
    nc.vector.memset(bd_f, 0.0)
    nc.vector.memset(bd_f[0:64, 0:1], 1.0)
    nc.vector.memset(bd_f[64:128, 1:2], 1.0)
    bd_r = consts.tile([128, 2], F32R)
    nc.vector.tensor_copy(out=bd_r, in_=bd_f)
    # bias matmuls ride in the same PSUM accumulation group as the bf16
    # GEMM chains, so their operands must be bf16 as well
    ones512_b = consts.tile([1, 512], BF16)
    nc.vector.memset(ones512_b, 1.0)
    ones128_b = consts.tile([1, 128], BF16)
    nc.vector.memset(ones128_b, 1.0)
    bqkr_t = consts.tile([1, 2 * FL], BF16)
    nc.scalar.dma_start(out=bqkr_t, in_=bqkr_d[:, :])
    bvr_t = consts.tile([1, FL], BF16)
    nc.scalar.dma_start(out=bvr_t, in_=bvr_d[:, :])

    oms_t = [consts.tile([128, 1], F32, name=f"oms{m}") for m in range(2)]
    for m in range(2):
        nc.scalar.dma_start(out=oms_t[m], in_=omsrep_d[m * 128:(m + 1) * 128, :])
    s_t = [consts.tile([128, 1], F32, name=f"sr{m}") for m in range(2)]
    for m in range(2):
        nc.scalar.dma_start(out=s_t[m], in_=srep_d[m * 128:(m + 1) * 128, :])
    alibi_t = [consts.tile([128, NK], F32, name=f"ali{h}") for h in range(HG)]
    for h in range(HG):
        nc.gpsimd.dma_start(out=alibi_t[h], in_=alibi_d[h, :, :])

    # persistent activation tiles (qT/kT rows 0:64 = head data, row 64 = aug)
    qT = [persist.tile([65, L], BF16, name=f"qT{h}") for h in range(HG)]
    kT = [persist.tile([65, L], BF16, name=f"kT{h}") for h in range(HG)]
    # kT row 64 = 8.0 (augmentation constant); rows 0:64 come from the GEMM
    for h in range(HG):
        nc.gpsimd.memset(kT[h][64:65, :], 8.0)

    hTp = ctx.enter_context(tc.tile_pool(name="hTp", bufs=1))
    hT = [hTp.tile([128, 4, L], BF16, name=f"hT{g}") for g in range(2)]
    wp = ctx.enter_context(tc.tile_pool(name="wp", bufs=1))
    wqk8 = wp.tile([128, 8, 2 * FL], BF16, name="wqk8")
    wqk_t = [wqk8[:, kc, :] for kc in range(8)]
    wv8 = wp.tile([128, 8, FL], BF16, name="wv8")
    wv_t = [wv8[:, kc, :] for kc in range(8)]
    wo2 = wp.tile([128, 2, DM], BF16, name="wo2")
    wo_t = [wo2[:, kc, :] for kc in range(2)]

    sqp = ctx.enter_context(tc.tile_pool(name="sqp", bufs=1))
    qn_bf = [sqp.tile([2, L], BF16, name=f"qn{p}") for p in range(2)]
    kmx = [sqp.tile([2, 4], F32, name=f"kmx{p}") for p in range(2)]

    vp = ctx.enter_context(tc.tile_pool(name="vp", bufs=1))
    v_sb = vp.tile([128, NLT, HG, 65], BF16)

    oTp = ctx.enter_context(tc.tile_pool(name="oTp", bufs=1))
    oT = [oTp.tile([128, L], BF16, name=f"oT{m}") for m in range(2)]

    wop = ctx.enter_context(tc.tile_pool(name="wop", bufs=1))
    g2b_t = wop.tile([128, DM], F32)
    nc.gpsimd.dma_start(out=g2b_t, in_=_bcast_ap(ln2g_d, 128, DM))
    b2b_t = wop.tile([128, DM], F32)
    nc.gpsimd.dma_start(out=b2b_t, in_=_bcast_ap(ln2b_d, 128, DM))
    g2b_bf = wop.tile([128, DM], BF16)
    nc.vector.tensor_copy(out=g2b_bf, in_=g2b_t)
    b2b_bf = wop.tile([128, DM], BF16)
    nc.vector.tensor_copy(out=b2b_bf, in_=b2b_t)

    ypart = [dram.tile([QB, DM], BF16, name=f"ypart{i}") for i in range(NCH)]
    yred = [dram.tile([QB // 4, DM], BF16, name=f"yred{i}") for i in range(NCH)]

    # working pools
    xp = ctx.enter_context(tc.tile_pool(name="xp", bufs=2))
    x4p = ctx.enter_context(tc.tile_pool(name="x4p", bufs=3))
    stp = ctx.enter_context(tc.tile_pool(name="stp", bufs=6))
    ktp = ctx.enter_context(tc.tile_pool(name="ktp", bufs=1))
    mtp = ctx.enter_context(tc.tile_pool(name="mtp", bufs=2))
    atp = ctx.enter_context(tc.tile_pool(name="atp", bufs=4))
    nrm = ctx.enter_context(tc.tile_pool(name="nrm", bufs=2))
    ysp = ctx.enter_context(tc.tile_pool(name="ysp", bufs=2))

    psW = ctx.enter_context(tc.tile_pool(name="psW", bufs=4, space="PSUM"))
    psO = ctx.enter_context(tc.tile_pool(name="psO", bufs=2, space="PSUM"))
    psv = ctx.enter_context(tc.tile_pool(name="psv", bufs=2, space="PSUM"))

    xr = xin.rearrange("(i j p) d -> i p j d", j=2, p=128)
    state = {"x4": None}
    kbcol = {}

    def ph1_lt(lt):
        # LN1 + PE transpose for one l-tile; x for a pair of l-tiles is
        # DMA'd at its head; weight loads are queued right after the first x.
        if True:
            if lt % 2 == 0:
                x4 = x4p.tile([128, 2, DM], F32, name="x4", tag="x4")
                nc.sync.dma_start(out=x4, in_=xr[lt // 2])
                state["x4"] = x4
                if lt == 0:
                    nc.sync.dma_start(
                        out=wqk8,
                        in_=wqk_d.rearrange("(c p) n -> p c n", p=128))
                    nc.sync.dma_start(
                        out=wv8,
                        in_=wv_d.rearrange("(c p) n -> p c n", p=128))
                    nc.sync.dma_start(
                        out=wo2,
                        in_=wo_d.rearrange("(c p) n -> p c n", p=128))
            x_t = state["x4"][:, lt % 2, :]
            st = stp.tile([128, 2, 6], F32)
            nc.vector.bn_stats(out=st[:, 0, :], in_=x_t[:, 0:512])
            nc.vector.bn_stats(out=st[:, 1, :], in_=x_t[:, 512:1024])
            mv = stp.tile([128, 2], F32)
            nc.vector.bn_aggr(out=mv, in_=st)
            # rstd = exp(-0.5*ln(var+eps)): stays in the Exp/Ln act set
            rstd = stp.tile([128, 1], F32)
            nc.scalar.activation(out=rstd, in_=mv[:, 1:2], func=AF.Ln,
                                 bias=eps_t, scale=1.0)
            nc.scalar.activation(out=rstd, in_=rstd, func=AF.Exp,
                                 bias=0.0, scale=-0.5)
            # LN apply on Pool (SBUF-only op — Pool cannot touch PSUM)
            h_t = xp.tile([128, DM], F32)
            nc.gpsimd.tensor_scalar(out=h_t, in0=x_t, scalar1=mv[:, 0:1],
                                    scalar2=rstd, op0=ALU.subtract,
                                    op1=ALU.mult)
            for g in range(2):
                pst = psW.tile([128, 512], F32, name="pst", tag="w")
                for j in range(4):
                    dc = 4 * g + j
                    nc.tensor.transpose(pst[:, j * 128:(j + 1) * 128],
                                        h_t[:, dc * 128:(dc + 1) * 128],
                                        ident)
                ceng = nc.scalar.copy if g == 0 else \
                    (lambda out, in_: nc.vector.tensor_copy(out=out, in_=in_))
                ceng(out=hT[g][:, :, lt * 128:(lt + 1) * 128],
                     in_=pst.rearrange("p (a b) -> p a b", a=4))

    def gemm_m(n, m):
        # QK GEMM for columns n*512..(n+1)*512, one head-pair m (0,1: q;
        # 2,3: k), plus the inline row-norm stats (for -M) and key smear.
        nsl = slice(n * 512, (n + 1) * 512)
        if True:
            pair = m % 2
            is_q = m < 2
            ps = psW.tile([128, 512], F32, name="psqk", tag="w")
            for kc in range(8):
                nc.tensor.matmul(
                    ps, wqk_t[kc][:, m * 128:(m + 1) * 128],
                    hT[kc // 4][:, kc % 4, nsl],
                    start=(kc == 0), stop=False)
            nc.tensor.matmul(ps, bqkr_t[:, m * 128:(m + 1) * 128],
                             ones512_b, start=False, stop=True)
            # row-norm statistics: sq = (x + b)^2 on ACT, then a blockdiag
            # column-sum -> per-head-pair norms
            sq_t = sqp.tile([128, 512], F32R, name="sq", tag="sq", bufs=2)
            nc.scalar.activation(out=sq_t, in_=ps, func=AF.Square,
                                 bias=0.0, scale=1.0)
            pw2 = psW.tile([128, 512], F32, name="pn2", tag="w")
            pn2 = pw2[0:2, :]
            nc.tensor.matmul(pn2, bd_r, sq_t, start=True, stop=True)
            if is_q:
                nc.scalar.copy(out=qn_bf[pair][:, nsl], in_=pn2)
            else:
                nc.vector.reduce_max(out=kmx[pair][:, n:n + 1],
                                     in_=pn2, axis=AX.X)
            for hh in range(2):
                h = pair * 2 + hh
                rows = slice(hh * 64, (hh + 1) * 64)
                if is_q:
                    nc.scalar.copy(out=qT[h][0:64, nsl], in_=ps[rows, :])
                    continue
                # k already biased: kT = k*(1-s); tmp = k*s; the shifted
                # add completes the smear per column block.  PSUM reads must
                # be DVE; the SBUF-only adds alternate DVE/Pool.
                veng = nc.vector
                aeng = nc.vector if hh == 1 else nc.gpsimd
                veng.tensor_scalar(
                    out=kT[h][0:64, nsl], in0=ps[rows, :],
                    scalar1=oms_t[pair][rows, :], scalar2=None,
                    op0=ALU.mult)
                tmp = ktp.tile([64, 512], BF16, name="ktmp", tag="ktmp",
                               bufs=3)
                veng.tensor_scalar(
                    out=tmp, in0=ps[rows, :],
                    scalar1=s_t[pair][rows, :], scalar2=None,
                    op0=ALU.mult)
                c0 = n * 512
                aeng.tensor_tensor(
                    out=kT[h][0:64, c0 + 1:c0 + 512],
                    in0=kT[h][0:64, c0 + 1:c0 + 512],
                    in1=tmp[:, 0:511], op=ALU.add)
                if n > 0:
                    aeng.tensor_tensor(
                        out=kT[h][0:64, c0:c0 + 1],
                        in0=kT[h][0:64, c0:c0 + 1],
                        in1=kbcol[h][:, 0:1], op=ALU.add)
                if n < 3:
                    bc = ktp.tile([64, 1], BF16, name=f"kb{h}",
                                  tag=f"kb{h}", bufs=2)
                    nc.gpsimd.tensor_copy(out=bc, in_=tmp[:, 511:512])
                    kbcol[h] = bc

    def emit_v_lt(lt):
        if True:
            ps = psv.tile([128, FL], F32, name="psv", tag="psv")
            for kc in range(8):
                nc.tensor.matmul(
                    ps, hT[kc // 4][:, kc % 4, lt * 128:(lt + 1) * 128],
                    wv_t[kc], start=(kc == 0), stop=False)
            nc.tensor.matmul(ps, ones128_b, bvr_t, start=False, stop=True)
            nc.scalar.copy(
                out=v_sb[:, lt, :, 0:64],
                in_=ps.rearrange("p (a b) -> p a b", a=HG))
            nc.vector.tensor_copy(
                out=v_sb[:, lt, :, 64:65],
                in_=onesvcol_b.rearrange("p (a b) -> p a b", a=HG))

    def emit_m_rows(qb):
        # -M = -(qn + kmax^2)/16 - relu(slope)*i over this chunk's queries;
        # kmax over n-tiles 0..qb only (all keys this chunk can see).
        qsl = slice(qb * QB, (qb + 1) * QB)
        for pair in range(2):
            aliq_c = mtp.tile([2, QB], BF16, name="aliqc", tag="aliqc")
            nc.scalar.dma_start(out=aliq_c,
                                in_=aliq_d[pair * 2:pair * 2 + 2, qsl])
            kms2 = mtp.tile([2, 1], F32, name="kms2", tag="kms2")
            nc.vector.reduce_max(out=kms2, in_=kmx[pair][:, 0:qb + 1],
                                 axis=AX.X)
            stag = mtp.tile([2, QB], BF16, name="stag", tag="stag")
            with nc.allow_low_precision(reason="-M guard tolerates bf16"):
                nc.vector.tensor_scalar(out=stag, in0=qn_bf[pair][:, qsl],
                                        scalar1=kms2, scalar2=-1.0 / 16.0,
                                        op0=ALU.add, op1=ALU.mult)
            nc.gpsimd.tensor_tensor(out=stag, in0=stag, in1=aliq_c,
                                    op=ALU.subtract)
            # row 0 is partition-aligned (engine copy); row 1 is not, so it
            # moves via a small SBUF-to-SBUF DMA on the Pool queue
            nc.vector.tensor_copy(out=qT[pair * 2][64:65, qsl],
                                  in_=stag[0:1, :])
            nc.sync.dma_start(out=qT[pair * 2 + 1][64:65, qsl],
                              in_=stag[1:2, :])

    def attn_chunk(qb, filler=()):
        # one software-pipelined stream over (head, k-block): QK+Exp run 3
        # items ahead of PV, crossing head boundaries, so neither PE nor ACT
        # ever drains; each head's normalize is emitted right after its
        # last PV.  `filler` closures (next iteration's transposes/GEMM/V,
        # previous chunk's proj/RS/LN2) are spread through the stream to
        # keep the PE dense and hot.
        qlo = qb * QB
        nkb = (qlo + QB) // 128
        ops_t = {}

        def emit_qk(h, kbi):
            kb = kbi * 128
            off = max(0, kb - qlo)
            sps = psW.tile([128, QB], F32, name="sps", tag="w")
            nc.tensor.matmul(sps[:, off:QB], kT[h][:, kb:kb + 128],
                             qT[h][:, qlo + off:qlo + QB],
                             start=True, stop=True)
            at = atp.tile([128, QB], BF16, name="at", tag="at")
            nc.scalar.activation(out=at[:, off:QB], in_=sps[:, off:QB],
                                 func=AF.Exp,
                                 bias=alibi_t[h][:, kbi:kbi + 1],
                                 scale=0.125)
            if kb >= qlo:
                nc.gpsimd.affine_select(
                    out=at[:, off:off + 128], in_=at[:, off:off + 128],
                    compare_op=ALU.is_ge, fill=0.0, base=0,
                    channel_multiplier=-1, pattern=[[1, 128]])
            return h, kbi, off, at

        def emit_pv(item):
            h, kbi, off, at = item
            if kbi == 0:
                ops_t[h] = psO.tile([65, QB], F32, name="ops", tag="ops")
            nc.tensor.matmul(ops_t[h][:, off:QB], v_sb[:, kbi, h, :],
                             at[:, off:QB],
                             start=(kbi == 0), stop=(kbi == nkb - 1))
            if kbi == nkb - 1:
                emit_norm(h)

        def emit_norm(h):
            ops = ops_t[h]
            dr_r = nrm.tile([1, QB], F32R, name="drr", tag="drr")
            with nc.allow_low_precision(reason="f32r is f32 bits"):
                nc.vector.reciprocal(out=dr_r, in_=ops[64:65, :])
            bps = psW.tile([128, QB], F32, name="bps", tag="w")
            nc.tensor.matmul(bps[0:64, :], ones64_r, dr_r,
                             start=True, stop=True)
            bsb = nrm.tile([64, QB], F32, name="bsb", tag="bsb")
            nc.vector.tensor_copy(out=bsb, in_=bps[0:64, :])
            r0 = (h % 2) * 64
            nc.vector.tensor_mul(out=oT[h // 2][r0:r0 + 64, qlo:qlo + QB],
                                 in0=ops[0:64, :], in1=bsb)

        filler = list(filler)
        n_items = HG * nkb
        done_f = 0
        pend = []
        idx = 0
        for h in range(HG):
            for kbi in range(nkb):
                pend.append(emit_qk(h, kbi))
                if len(pend) > 3:
                    emit_pv(pend.pop(0))
                idx += 1
                want = (idx * len(filler)) // n_items
                while done_f < want:
                    filler[done_f]()
                    done_f += 1
        for item in pend:
            emit_pv(item)
        while done_f < len(filler):
            filler[done_f]()
            done_f += 1

    def proj_lt(qb, j):
        lt = qb * (QB // 128) + j
        ysb = ysp.tile([128, DM], BF16, name="ysb", tag="ysb")
        for n2 in range(2):
            ps = psW.tile([128, 512], F32, name="psy", tag="w")
            for kc in range(2):
                nc.tensor.matmul(ps, oT[kc][:, lt * 128:(lt + 1) * 128],
                                 wo_t[kc][:, n2 * 512:(n2 + 1) * 512],
                                 start=(kc == 0), stop=(kc == 1))
            if n2 == 0:
                nc.scalar.copy(out=ysb[:, 0:512], in_=ps)
            else:
                nc.vector.tensor_copy(out=ysb[:, 512:1024], in_=ps)
        nc.sync.dma_start(out=ypart[qb][j * 128:(j + 1) * 128, :], in_=ysb)

    def rs_chunk(qb):
        nc.gpsimd.collective_compute(
            "ReduceScatter", ALU.add,
            replica_groups=[[0, 1, 2, 3], [4, 5, 6, 7]],
            ins=[ypart[qb][:, :]], outs=[yred[qb][:, :]])

    def ln2_chunk(qb):
        y_t = ysp.tile([128, DM], BF16, name="y2t", tag="y2t")
        nc.sync.dma_start(out=y_t, in_=yred[qb][:, :])
        st = ysp.tile([128, 2, 6], F32, name="st2", tag="st2")
        nc.vector.bn_stats(out=st[:, 0, :], in_=y_t[:, 0:512])
        nc.vector.bn_stats(out=st[:, 1, :], in_=y_t[:, 512:1024])
        mv = ysp.tile([128, 2], F32, name="mv2", tag="mv2")
        nc.vector.bn_aggr(out=mv, in_=st)
        rstd = ysp.tile([128, 1], F32, name="rstd2", tag="rstd2")
        nc.scalar.activation(out=rstd, in_=mv[:, 1:2], func=AF.Ln,
                             bias=eps_t, scale=1.0)
        nc.scalar.activation(out=rstd, in_=rstd, func=AF.Exp,
                             bias=0.0, scale=-0.5)
        xh = ysp.tile([128, DM], BF16, name="xh", tag="xh")
        nc.vector.tensor_scalar(out=xh, in0=y_t, scalar1=mv[:, 0:1],
                                scalar2=rstd, op0=ALU.subtract, op1=ALU.mult)
        nc.vector.tensor_tensor(out=xh, in0=xh, in1=g2b_bf, op=ALU.mult)
        o_t = ysp.tile([128, DM], F32, name="o2t", tag="o2t", bufs=1)
        nc.vector.tensor_tensor(out=o_t, in0=xh, in1=b2b_bf, op=ALU.add)
        nc.sync.dma_start(out=out_d[qb * 128:(qb + 1) * 128, :], in_=o_t)

    # ---- fused emission: iteration 0 is emitted straight; afterwards each
    # chunk's attention stream carries the NEXT iteration's LN/transpose/
    # GEMM/V and the PREVIOUS chunk's proj/RS/LN2 as interleaved filler so
    # the PE stream stays dense (and hot) end to end. ----
    def mk(f, *a):
        return lambda: f(*a)

    import os
    no_fill = os.environ.get("KNOFILL", "0") == "1"

    _mark('g0', nc)
    for lt in range(4):
        ph1_lt(lt)
    for m in range(4):
        gemm_m(0, m)
    for lt in range(4):
        emit_v_lt(lt)
    emit_m_rows(0)
    for n in range(4):
        _mark(f'a{n}', nc)
        filler = []
        if n > 0:
            for j in range(4):
                filler.append(mk(proj_lt, n - 1, j))
            filler.append(mk(rs_chunk, n - 1))
        if n < 3:
            for j4 in range(4):
                filler.append(mk(ph1_lt, 4 * (n + 1) + j4))
            for m in range(4):
                filler.append(mk(gemm_m, n + 1, m))
            for j4 in range(4):
                filler.append(mk(emit_v_lt, 4 * (n + 1) + j4))
            filler.append(mk(emit_m_rows, n + 1))
        if n > 1:
            filler.append(mk(ln2_chunk, n - 2))
        if no_fill:
            for f in filler:
                f()
            attn_chunk(n, ())
        else:
            attn_chunk(n, filler)
    _mark('tail', nc)
    for j in range(4):
        proj_lt(3, j)
    rs_chunk(3)
    ln2_chunk(2)
    ln2_chunk(3)


def _prep_inputs(x, ln1_g, ln1_b, in_w, out_w, ln2_g, ln2_b, slopes, smear):
    """Slice/transpose per-core views of the weights (host-side marshaling)."""
    x = np.asarray(x, np.float32)
    in_w = np.asarray(in_w, np.float32)
    out_w = np.asarray(out_w, np.float32)
    ln1_g = np.asarray(ln1_g, np.float32)
    ln1_b = np.asarray(ln1_b, np.float32)
    slopes = np.asarray(slopes, np.float32)
    smear = np.asarray(smear, np.float32)
    w_eff = in_w * ln1_g[None, :]
    qkvb = in_w @ ln1_b
    sig = 1.0 / (1.0 + np.exp(-smear))
    bf = ml_dtypes.bfloat16
    in_maps = []
    for c in range(NCORES):
        b, hg = c // 4, c % 4
        f0 = FL * hg
        wq = w_eff[f0:f0 + FL]
        wk = w_eff[DM + f0:DM + f0 + FL]
        wv = w_eff[2 * DM + f0:2 * DM + f0 + FL]
        sl4 = slopes[4 * hg:4 * hg + 4]
        p = np.arange(128, dtype=np.float32)
        kbv = np.arange(NK, dtype=np.float32) * 128.0
        alibi = sl4[:, None, None] * (kbv[None, None, :] + p[None, :, None])
        aliq = np.maximum(sl4, 0.0)[:, None] * np.arange(L, dtype=np.float32)[None, :]
        in_maps.append({
            "xin": np.ascontiguousarray(x[b]),
            "wqk": np.ascontiguousarray(
                np.concatenate([wq, wk], 0).T).astype(bf),
            "wv": np.ascontiguousarray(wv.T).astype(bf),
            "wo": np.ascontiguousarray(out_w[:, f0:f0 + FL].T).astype(bf),
            "bqkr": np.ascontiguousarray(
                np.concatenate([qkvb[f0:f0 + FL],
                                qkvb[DM + f0:DM + f0 + FL]])[None, :]).astype(bf),
            "bvr": np.ascontiguousarray(
                qkvb[2 * DM + f0:2 * DM + f0 + FL][None, :]).astype(bf),
            "srep": np.repeat(sig[4 * hg:4 * hg + 4], 64)[:, None].astype(np.float32),
            "omsrep": np.repeat(1.0 - sig[4 * hg:4 * hg + 4], 64)[:, None].astype(np.float32),
            "alibi": np.ascontiguousarray(alibi.astype(np.float32)),
            "aliq": np.ascontiguousarray(aliq.astype(np.float32)).astype(bf),
            "ln2g": np.asarray(ln2_g, np.float32),
            "ln2b": np.asarray(ln2_b, np.float32),
        })
    return in_maps


def kernel(**inputs):
    if "nc" not in _CACHE:
        _CACHE["nc"] = _build_program()
    nc = _CACHE["nc"]
    in_maps = _prep_inputs(**inputs)
    res = run_bass_kernel_spmd(nc, in_maps, core_ids=list(range(NCORES)))
    out = np.empty((B, L, DM), np.float32)
    for c in range(NCORES):
        b, hg = c // 4, c % 4
        r = res.results[c]["out"]
        for qb in range(NCH):
            g0 = qb * QB + hg * (QB // 4)
            out[b, g0:g0 + QB // 4, :] = r[qb * (QB // 4):(qb + 1) * (QB // 4), :]
    return out


# revision 69
# speedup vs baseline: 1.1931x; 1.1207x over previous
"""Trainium2 Bass kernel for nn_Attention_49709951484392 (causal attention
block: LN1 -> QKV -> key smearing -> causal attention with learned ALiBi ->
out-proj -> LN2), sharded over 8 NeuronCores.

Sharding: core c handles batch c//4 and head-group c%4 (4 of 16 heads).
Out-projection partial sums are ReduceScatter'ed over each batch's 4-core
group; each core then runs LN2 on its 512-row slice of the output.

Attention runs in transposed orientation S^T[k, q] so that:
  - the ALiBi term slope*j (j = key position) is a per-partition bias folded
    into the Exp activation,
  - the per-query shift M_i (softmax overflow guard) is folded into the QK
    matmul by augmenting kT with a constant row (8.0) and qT with a row
    holding -M_i (65-dim contraction),
  - the softmax denominator is produced by the PV matmul via an extra ones
    column appended to V (row 64 of the PV output),
so no transposes of the attention matrix are needed.  M_i is the bound
(|q_i|^2 + max_j|k_j|^2)/16 + relu(slope)*i >= max_j (q_i.k_j/8 + slope*j),
computed with one augmented column-sum matmul per head; kmax is taken over
only the key n-tiles a query chunk can see, so chunk n's attention starts
right after GEMM n-tile n.

The emission is one fused loop over the four 512-column n-tiles:
  LN1+transpose group n -> QK GEMM n -> V GEMM group n -> -M rows chunk n ->
  attention chunk n (out-proj of chunk n-1 interleaved per head) ->
  ReduceScatter chunk n-1 -> LN2 chunk n-2
which keeps the in-order PE stream dense and hides the collectives.

Activations/weights are bf16 on the PE (f32 PSUM accumulation); the act
table dict is reordered so Exp/Ln/Copy/Square all live in one function set
(no LoadActFuncSet churn).
"""
import sys

import numpy as np
import ml_dtypes

sys.path.insert(0, "/opt/trn_rl_repo")

import concourse.bacc as bacc
import concourse.bass as bass
import concourse.mybir as mybir
import concourse.tile as tile
from concourse.bass_utils import run_bass_kernel_spmd
from concourse.masks import make_identity

F32 = mybir.dt.float32
F32R = mybir.dt.float32r
BF16 = mybir.dt.bfloat16
AF = mybir.ActivationFunctionType
ALU = mybir.AluOpType
AX = mybir.AxisListType

HEADS = 16
DH = 64
DM = 1024
B, L = 2, 2048
EPS = 1e-5
NCORES = 8
HG = 4          # heads per core
FL = HG * DH    # local feature width (256)
QB = 512        # query chunk == n-tile width
NK = L // 128   # key blocks of 128
NLT = L // 128  # l-tiles
NCH = L // QB   # chunks (4)

_CACHE = {}
PHASE_MARKS = []


def _mark(name, nc):
    ids = []
    for k in nc.inst_map.keys():
        if isinstance(k, str) and k.startswith("I-"):
            try:
                ids.append(int(k.split("-")[1]))
            except ValueError:
                pass
    PHASE_MARKS.append((name, max(ids) if ids else 0))


def _patch_act_tables():
    """Put the set containing both Exp and Ln first so the act-table pass
    assigns every activation in this kernel to one set (zero reloads)."""
    import concourse.hw_specs as hws
    if getattr(bacc, "_act_tables_patched", False):
        return
    orig = hws.get_activation_tables

    def constrained(module_arch):
        # Keep canonical set order/ids (the runtime keys tables by id), but
        # hide Exp/Ln from every set except the one containing both, so the
        # table-load pass assigns all our activations to that single set.
        t = orig(module_arch)
        import concourse.mybir as mb
        AFt = mb.ActivationFunctionType
        want = {AFt.Exp, AFt.Ln, AFt.Copy, AFt.Square}
        best = None
        for name, funcs in t.items():
            if want <= funcs:
                best = name
                break
        if best is None:
            return t
        out = {}
        for name, funcs in t.items():
            if name == best:
                out[name] = funcs
            else:
                out[name] = funcs - {AFt.Exp, AFt.Ln}
        return out

    bacc.get_activation_tables = constrained
    bacc._act_tables_patched = True


def _build_program():
    _patch_act_tables()
    nc = bacc.Bacc()
    xin = nc.declare_dram_parameter("xin", [L, DM], F32, isOutput=False)
    wqk_d = nc.declare_dram_parameter("wqk", [DM, 2 * FL], BF16, isOutput=False)
    wv_d = nc.declare_dram_parameter("wv", [DM, FL], BF16, isOutput=False)
    wo_d = nc.declare_dram_parameter("wo", [FL, DM], BF16, isOutput=False)
    bqkr_d = nc.declare_dram_parameter("bqkr", [1, 2 * FL], BF16, isOutput=False)
    bvr_d = nc.declare_dram_parameter("bvr", [1, FL], BF16, isOutput=False)
    srep_d = nc.declare_dram_parameter("srep", [FL, 1], F32, isOutput=False)
    omsrep_d = nc.declare_dram_parameter("omsrep", [FL, 1], F32, isOutput=False)
    alibi_d = nc.declare_dram_parameter("alibi", [HG, 128, NK], F32, isOutput=False)
    aliq_d = nc.declare_dram_parameter("aliq", [HG, L], BF16, isOutput=False)
    ln2g_d = nc.declare_dram_parameter("ln2g", [DM], F32, isOutput=False)
    ln2b_d = nc.declare_dram_parameter("ln2b", [DM], F32, isOutput=False)
    out_d = nc.declare_dram_parameter("out", [L // 4, DM], F32, isOutput=True)

    from contextlib import ExitStack
    with tile.TileContext(nc) as tc, ExitStack() as ctx:
        _emit(ctx, nc, tc, xin, wqk_d, wv_d, wo_d, bqkr_d, bvr_d,
              srep_d, omsrep_d, alibi_d, aliq_d, ln2g_d, ln2b_d, out_d)
    nc.compile()
    return nc


def _bcast_ap(handle, parts, free):
    ap = handle[:]
    return bass.AP(tensor=ap.tensor, offset=0, ap=[[0, parts], [1, free]])


def _emit(ctx, nc, tc, xin, wqk_d, wv_d, wo_d, bqkr_d, bvr_d,
          srep_d, omsrep_d, alibi_d, aliq_d, ln2g_d, ln2b_d, out_d):
    consts = ctx.enter_context(tc.tile_pool(name="consts", bufs=1))
    persist = ctx.enter_context(tc.tile_pool(name="persist", bufs=1))
    dram = ctx.enter_context(tc.tile_pool(name="dram", bufs=1, space="DRAM"))

    ident = consts.tile([128, 128], F32)
    make_identity(nc, ident)
    identb = consts.tile([128, 128], BF16)
    nc.vector.tensor_copy(out=identb, in_=ident)
    eps_t = consts.tile([128, 1], F32)
    nc.vector.memset(eps_t, EPS)
    ones64_f = consts.tile([1, 64], F32)
    nc.vector.memset(ones64_f, 1.0)
    ones64_r = consts.tile([1, 64], F32R)
    nc.vector.tensor_copy(out=ones64_r, in_=ones64_f)
    onesvcol_b = consts.tile([128, HG], BF16)
    nc.vector.memset(onesvcol_b, 1.0)
    bd_f = consts.tile([128, 2], F32)
    nc.vector.memset(bd_f, 0.0)
    nc.vector.memset(bd_f[0:64, 0:1], 1.0)
    nc.vector.memset(bd_f[64:128, 1:2], 1.0)
    bd_r = consts.tile([128, 2], F32R)
    nc.vector.tensor_copy(out=bd_r, in_=bd_f)
    # bias matmuls ride in the same PSUM accumulation group as the bf16
    # GEMM chains, so their operands must be bf16 as well
    ones512_b = consts.tile([1, 512], BF16)
    nc.vector.memset(ones512_b, 1.0)
    ones128_b = consts.tile([1, 128], BF16)
    nc.vector.memset(ones128_b, 1.0)
    bqkr_t = consts.tile([1, 2 * FL], BF16)
    nc.scalar.dma_start(out=bqkr_t, in_=bqkr_d[:, :])
    bvr_t = consts.tile([1, FL], BF16)
    nc.scalar.dma_start(out=bvr_t, in_=bvr_d[:, :])

    oms_t = [consts.tile([128, 1], F32, name=f"oms{m}") for m in range(2)]
    for m in range(2):
        nc.scalar.dma_start(out=oms_t[m], in_=omsrep_d[m * 128:(m + 1) * 128, :])
    s_t = [consts.tile([128, 1], F32, name=f"sr{m}") for m in range(2)]
    for m in range(2):
        nc.scalar.dma_start(out=s_t[m], in_=srep_d[m * 128:(m + 1) * 128, :])
    alibi_t = [consts.tile([128, NK], F32, name=f"ali{h}") for h in range(HG)]
    for h in range(HG):
        nc.scalar.dma_start(out=alibi_t[h], in_=alibi_d[h, :, :])

    # persistent activation tiles (qT/kT rows 0:64 = head data, row 64 = aug)
    qT = [persist.tile([65, L], BF16, name=f"qT{h}") for h in range(HG)]
    kT = [persist.tile([65, L], BF16, name=f"kT{h}") for h in range(HG)]

    hTp = ctx.enter_context(tc.tile_pool(name="hTp", bufs=1))
    hT = [hTp.tile([128, 4, L], BF16, name=f"hT{g}") for g in range(2)]
    wp = ctx.enter_context(tc.tile_pool(name="wp", bufs=1))
    wqk8 = wp.tile([128, 8, 2 * FL], BF16, name="wqk8")
    wqk_t = [wqk8[:, kc, :] for kc in range(8)]
    wv8 = wp.tile([128, 8, FL], BF16, name="wv8")
    wv_t = [wv8[:, kc, :] for kc in range(8)]
    wo2 = wp.tile([128, 2, DM], BF16, name="wo2")
    wo_t = [wo2[:, kc, :] for kc in range(2)]

    sqp = ctx.enter_context(tc.tile_pool(name="sqp", bufs=1))
    qn_bf = [sqp.tile([2, L], BF16, name=f"qn{p}") for p in range(2)]
    kmx = [sqp.tile([2, 4], F32, name=f"kmx{p}") for p in range(2)]

    vp = ctx.enter_context(tc.tile_pool(name="vp", bufs=1))
    v_sb = vp.tile([128, NLT, HG, 65], BF16)

    oTp = ctx.enter_context(tc.tile_pool(name="oTp", bufs=1))
    oT = [oTp.tile([128, L], BF16, name=f"oT{m}") for m in range(2)]

    wop = ctx.enter_context(tc.tile_pool(name="wop", bufs=1))
    g2b_t = wop.tile([128, DM], F32)
    nc.scalar.dma_start(out=g2b_t, in_=_bcast_ap(ln2g_d, 128, DM))
    b2b_t = wop.tile([128, DM], F32)
    nc.scalar.dma_start(out=b2b_t, in_=_bcast_ap(ln2b_d, 128, DM))
    g2b_bf = wop.tile([128, DM], BF16)
    nc.vector.tensor_copy(out=g2b_bf, in_=g2b_t)
    b2b_bf = wop.tile([128, DM], BF16)
    nc.vector.tensor_copy(out=b2b_bf, in_=b2b_t)

    ypart = [dram.tile([QB, DM], BF16, name=f"ypart{i}") for i in range(NCH)]
    yred = [dram.tile([QB // 4, DM], BF16, name=f"yred{i}") for i in range(NCH)]

    # working pools
    xp = ctx.enter_context(tc.tile_pool(name="xp", bufs=2))
    x4p = ctx.enter_context(tc.tile_pool(name="x4p", bufs=4))
    stp = ctx.enter_context(tc.tile_pool(name="stp", bufs=6))
    ktp = ctx.enter_context(tc.tile_pool(name="ktp", bufs=1))
    mtp = ctx.enter_context(tc.tile_pool(name="mtp", bufs=2))
    atp = ctx.enter_context(tc.tile_pool(name="atp", bufs=4))
    nrm = ctx.enter_context(tc.tile_pool(name="nrm", bufs=2))
    ysp = ctx.enter_context(tc.tile_pool(name="ysp", bufs=2))

    psW = ctx.enter_context(tc.tile_pool(name="psW", bufs=4, space="PSUM"))
    psO = ctx.enter_context(tc.tile_pool(name="psO", bufs=2, space="PSUM"))
    psv = ctx.enter_context(tc.tile_pool(name="psv", bufs=2, space="PSUM"))

    xr = xin.rearrange("(i j p) d -> i p j d", j=2, p=128)
    state = {"x4": None}
    kbcol = {}

    def ph1_lt(lt):
        # LN1 + PE transpose for one l-tile; x for a pair of l-tiles is
        # DMA'd at its head; weight loads are queued right after the first x.
        if True:
            if lt % 2 == 0:
                x4 = x4p.tile([128, 2, DM], F32, name="x4", tag="x4")
                nc.sync.dma_start(out=x4, in_=xr[lt // 2])
                state["x4"] = x4
                if lt == 0:
                    nc.sync.dma_start(
                        out=wqk8,
                        in_=wqk_d.rearrange("(c p) n -> p c n", p=128))
                    nc.sync.dma_start(
                        out=wv8,
                        in_=wv_d.rearrange("(c p) n -> p c n", p=128))
                    nc.sync.dma_start(
                        out=wo2,
                        in_=wo_d.rearrange("(c p) n -> p c n", p=128))
            x_t = state["x4"][:, lt % 2, :]
            st = stp.tile([128, 2, 6], F32)
            nc.vector.bn_stats(out=st[:, 0, :], in_=x_t[:, 0:512])
            nc.vector.bn_stats(out=st[:, 1, :], in_=x_t[:, 512:1024])
            mv = stp.tile([128, 2], F32)
            nc.vector.bn_aggr(out=mv, in_=st)
            # rstd = exp(-0.5*ln(var+eps)): stays in the Exp/Ln act set
            rstd = stp.tile([128, 1], F32)
            nc.scalar.activation(out=rstd, in_=mv[:, 1:2], func=AF.Ln,
                                 bias=eps_t, scale=1.0)
            nc.scalar.activation(out=rstd, in_=rstd, func=AF.Exp,
                                 bias=0.0, scale=-0.5)
            # LN apply on Pool (SBUF-only op — Pool cannot touch PSUM)
            h_t = xp.tile([128, DM], F32)
            nc.gpsimd.tensor_scalar(out=h_t, in0=x_t, scalar1=mv[:, 0:1],
                                    scalar2=rstd, op0=ALU.subtract,
                                    op1=ALU.mult)
            for g in range(2):
                pst = psW.tile([128, 512], F32, name="pst", tag="w")
                for j in range(4):
                    dc = 4 * g + j
                    nc.tensor.transpose(pst[:, j * 128:(j + 1) * 128],
                                        h_t[:, dc * 128:(dc + 1) * 128],
                                        ident)
                ceng = nc.scalar.copy if g == 0 else \
                    (lambda out, in_: nc.vector.tensor_copy(out=out, in_=in_))
                ceng(out=hT[g][:, :, lt * 128:(lt + 1) * 128],
                     in_=pst.rearrange("p (a b) -> p a b", a=4))

    def gemm_m(n, m):
        # QK GEMM for columns n*512..(n+1)*512, one head-pair m (0,1: q;
        # 2,3: k), plus the inline row-norm stats (for -M) and key smear.
        nsl = slice(n * 512, (n + 1) * 512)
        if True:
            pair = m % 2
            is_q = m < 2
            ps = psW.tile([128, 512], F32, name="psqk", tag="w")
            for kc in range(8):
                nc.tensor.matmul(
                    ps, wqk_t[kc][:, m * 128:(m + 1) * 128],
                    hT[kc // 4][:, kc % 4, nsl],
                    start=(kc == 0), stop=False)
            nc.tensor.matmul(ps, bqkr_t[:, m * 128:(m + 1) * 128],
                             ones512_b, start=False, stop=True)
            # row-norm statistics: sq = (x + b)^2 on ACT, then a blockdiag
            # column-sum -> per-head-pair norms
            sq_t = sqp.tile([128, 512], F32R, name="sq", tag="sq", bufs=2)
            nc.scalar.activation(out=sq_t, in_=ps, func=AF.Square,
                                 bias=0.0, scale=1.0)
            pw2 = psW.tile([128, 512], F32, name="pn2", tag="w")
            pn2 = pw2[0:2, :]
            nc.tensor.matmul(pn2, bd_r, sq_t, start=True, stop=True)
            if is_q:
                nc.scalar.copy(out=qn_bf[pair][:, nsl], in_=pn2)
            else:
                nc.vector.reduce_max(out=kmx[pair][:, n:n + 1],
                                     in_=pn2, axis=AX.X)
            for hh in range(2):
                h = pair * 2 + hh
                rows = slice(hh * 64, (hh + 1) * 64)
                if is_q:
                    nc.scalar.copy(out=qT[h][0:64, nsl], in_=ps[rows, :])
                    continue
                # k already biased: kT = k*(1-s); tmp = k*s; the shifted
                # add completes the smear per column block.  PSUM reads must
                # be DVE; the SBUF-only adds alternate DVE/Pool.
                veng = nc.vector
                aeng = nc.vector if hh == 1 else nc.gpsimd
                veng.tensor_scalar(
                    out=kT[h][0:64, nsl], in0=ps[rows, :],
                    scalar1=oms_t[pair][rows, :], scalar2=None,
                    op0=ALU.mult)
                tmp = ktp.tile([64, 512], BF16, name="ktmp", tag="ktmp",
                               bufs=3)
                veng.tensor_scalar(
                    out=tmp, in0=ps[rows, :],
                    scalar1=s_t[pair][rows, :], scalar2=None,
                    op0=ALU.mult)
                c0 = n * 512
                aeng.tensor_tensor(
                    out=kT[h][0:64, c0 + 1:c0 + 512],
                    in0=kT[h][0:64, c0 + 1:c0 + 512],
                    in1=tmp[:, 0:511], op=ALU.add)
                if n > 0:
                    aeng.tensor_tensor(
                        out=kT[h][0:64, c0:c0 + 1],
                        in0=kT[h][0:64, c0:c0 + 1],
                        in1=kbcol[h][:, 0:1], op=ALU.add)
                if n < 3:
                    bc = ktp.tile([64, 1], BF16, name=f"kb{h}",
                                  tag=f"kb{h}", bufs=2)
                    nc.gpsimd.tensor_copy(out=bc, in_=tmp[:, 511:512])
                    kbcol[h] = bc

    def emit_v_lt(lt):
        if True:
            ps = psv.tile([128, FL], F32, name="psv", tag="psv")
            for kc in range(8):
                nc.tensor.matmul(
                    ps, hT[kc // 4][:, kc % 4, lt * 128:(lt + 1) * 128],
                    wv_t[kc], start=(kc == 0), stop=False)
            nc.tensor.matmul(ps, ones128_b, bvr_t, start=False, stop=True)
            nc.scalar.copy(
                out=v_sb[:, lt, :, 0:64],
                in_=ps.rearrange("p (a b) -> p a b", a=HG))
            nc.vector.tensor_copy(
                out=v_sb[:, lt, :, 64:65],
                in_=onesvcol_b.rearrange("p (a b) -> p a b", a=HG))

    def emit_m_rows(qb):
        # -M = -(qn + kmax^2)/16 - relu(slope)*i over this chunk's queries;
        # kmax over n-tiles 0..qb only (all keys this chunk can see).
        qsl = slice(qb * QB, (qb + 1) * QB)
        for pair in range(2):
            aliq_c = mtp.tile([2, QB], BF16, name="aliqc", tag="aliqc")
            nc.scalar.dma_start(out=aliq_c,
                                in_=aliq_d[pair * 2:pair * 2 + 2, qsl])
            kms2 = mtp.tile([2, 1], F32, name="kms2", tag="kms2")
            nc.vector.reduce_max(out=kms2, in_=kmx[pair][:, 0:qb + 1],
                                 axis=AX.X)
            stag = mtp.tile([2, QB], BF16, name="stag", tag="stag")
            with nc.allow_low_precision(reason="-M guard tolerates bf16"):
                nc.vector.tensor_scalar(out=stag, in0=qn_bf[pair][:, qsl],
                                        scalar1=kms2, scalar2=-1.0 / 16.0,
                                        op0=ALU.add, op1=ALU.mult)
            nc.gpsimd.tensor_tensor(out=stag, in0=stag, in1=aliq_c,
                                    op=ALU.subtract)
            # row 0 is partition-aligned (engine copy); row 1 is not, so it
            # moves via a small SBUF-to-SBUF DMA on the Pool queue
            nc.vector.tensor_copy(out=qT[pair * 2][64:65, qsl],
                                  in_=stag[0:1, :])
            nc.sync.dma_start(out=qT[pair * 2 + 1][64:65, qsl],
                              in_=stag[1:2, :])

    def attn_chunk(qb, filler=()):
        # one software-pipelined stream over (head, k-block): QK+Exp run 3
        # items ahead of PV, crossing head boundaries, so neither PE nor ACT
        # ever drains; each head's normalize is emitted right after its
        # last PV.  `filler` closures (next iteration's transposes/GEMM/V,
        # previous chunk's proj/RS/LN2) are spread through the stream to
        # keep the PE dense and hot.
        qlo = qb * QB
        nkb = (qlo + QB) // 128
        ops_t = {}

        def emit_qk(h, kbi):
            kb = kbi * 128
            off = max(0, kb - qlo)
            sps = psW.tile([128, QB], F32, name="sps", tag="w")
            nc.tensor.matmul(sps[:, off:QB], kT[h][:, kb:kb + 128],
                             qT[h][:, qlo + off:qlo + QB],
                             start=True, stop=True)
            at = atp.tile([128, QB], BF16, name="at", tag="at")
            nc.scalar.activation(out=at[:, off:QB], in_=sps[:, off:QB],
                                 func=AF.Exp,
                                 bias=alibi_t[h][:, kbi:kbi + 1],
                                 scale=0.125)
            if kb >= qlo:
                nc.gpsimd.affine_select(
                    out=at[:, off:off + 128], in_=at[:, off:off + 128],
                    compare_op=ALU.is_ge, fill=0.0, base=0,
                    channel_multiplier=-1, pattern=[[1, 128]])
            return h, kbi, off, at

        def emit_pv(item):
            h, kbi, off, at = item
            if kbi == 0:
                ops_t[h] = psO.tile([65, QB], F32, name="ops", tag="ops")
            nc.tensor.matmul(ops_t[h][:, off:QB], v_sb[:, kbi, h, :],
                             at[:, off:QB],
                             start=(kbi == 0), stop=(kbi == nkb - 1))
            if kbi == nkb - 1:
                emit_norm(h)

        def emit_norm(h):
            ops = ops_t[h]
            dr_r = nrm.tile([1, QB], F32R, name="drr", tag="drr")
            with nc.allow_low_precision(reason="f32r is f32 bits"):
                nc.vector.reciprocal(out=dr_r, in_=ops[64:65, :])
            bps = psW.tile([128, QB], F32, name="bps", tag="w")
            nc.tensor.matmul(bps[0:64, :], ones64_r, dr_r,
                             start=True, stop=True)
            bsb = nrm.tile([64, QB], F32, name="bsb", tag="bsb")
            nc.vector.tensor_copy(out=bsb, in_=bps[0:64, :])
            r0 = (h % 2) * 64
            nc.vector.tensor_mul(out=oT[h // 2][r0:r0 + 64, qlo:qlo + QB],
                                 in0=ops[0:64, :], in1=bsb)

        filler = list(filler)
        n_items = HG * nkb
        done_f = 0
        pend = []
        idx = 0
        for h in range(HG):
            for kbi in range(nkb):
                pend.append(emit_qk(h, kbi))
                if len(pend) > 3:
                    emit_pv(pend.pop(0))
                idx += 1
                want = (idx * len(filler)) // n_items
                while done_f < want:
                    filler[done_f]()
                    done_f += 1
        for item in pend:
            emit_pv(item)
        while done_f < len(filler):
            filler[done_f]()
            done_f += 1

    def proj_lt(qb, j):
        lt = qb * (QB // 128) + j
        ysb = ysp.tile([128, DM], BF16, name="ysb", tag="ysb")
        for n2 in range(2):
            ps = psW.tile([128, 512], F32, name="psy", tag="w")
            for kc in range(2):
                nc.tensor.matmul(ps, oT[kc][:, lt * 128:(lt + 1) * 128],
                                 wo_t[kc][:, n2 * 512:(n2 + 1) * 512],
                                 start=(kc == 0), stop=(kc == 1))
            nc.vector.tensor_copy(out=ysb[:, n2 * 512:(n2 + 1) * 512],
                                  in_=ps)
        nc.sync.dma_start(out=ypart[qb][j * 128:(j + 1) * 128, :], in_=ysb)

    def rs_chunk(qb):
        nc.gpsimd.collective_compute(
            "ReduceScatter", ALU.add,
            replica_groups=[[0, 1, 2, 3], [4, 5, 6, 7]],
            ins=[ypart[qb][:, :]], outs=[yred[qb][:, :]])

    def ln2_chunk(qb):
        y_t = ysp.tile([128, DM], BF16, name="y2t", tag="y2t")
        nc.sync.dma_start(out=y_t, in_=yred[qb][:, :])
        st = ysp.tile([128, 2, 6], F32, name="st2", tag="st2")
        nc.vector.bn_stats(out=st[:, 0, :], in_=y_t[:, 0:512])
        nc.vector.bn_stats(out=st[:, 1, :], in_=y_t[:, 512:1024])
        mv = ysp.tile([128, 2], F32, name="mv2", tag="mv2")
        nc.vector.bn_aggr(out=mv, in_=st)
        rstd = ysp.tile([128, 1], F32, name="rstd2", tag="rstd2")
        nc.scalar.activation(out=rstd, in_=mv[:, 1:2], func=AF.Ln,
                             bias=eps_t, scale=1.0)
        nc.scalar.activation(out=rstd, in_=rstd, func=AF.Exp,
                             bias=0.0, scale=-0.5)
        xh = ysp.tile([128, DM], BF16, name="xh", tag="xh")
        nc.vector.tensor_scalar(out=xh, in0=y_t, scalar1=mv[:, 0:1],
                                scalar2=rstd, op0=ALU.subtract, op1=ALU.mult)
        nc.vector.tensor_tensor(out=xh, in0=xh, in1=g2b_bf, op=ALU.mult)
        o_t = ysp.tile([128, DM], F32, name="o2t", tag="o2t", bufs=1)
        nc.vector.tensor_tensor(out=o_t, in0=xh, in1=b2b_bf, op=ALU.add)
        nc.sync.dma_start(out=out_d[qb * 128:(qb + 1) * 128, :], in_=o_t)

    # ---- fused emission: iteration 0 is emitted straight; afterwards each
    # chunk's attention stream carries the NEXT iteration's LN/transpose/
    # GEMM/V and the PREVIOUS chunk's proj/RS/LN2 as interleaved filler so
    # the PE stream stays dense (and hot) end to end. ----
    def mk(f, *a):
        return lambda: f(*a)

    import os
    no_fill = os.environ.get("KNOFILL", "0") == "1"

    _mark('g0', nc)
    for lt in range(4):
        ph1_lt(lt)
    # kT row 64 = 8.0 (augmentation constant); emitted after the first LN
    # applies so the Pool queue isn't clogged at t=0
    for h in range(HG):
        nc.gpsimd.memset(kT[h][64:65, :], 8.0)
    for m in range(4):
        gemm_m(0, m)
    for lt in range(4):
        emit_v_lt(lt)
    emit_m_rows(0)
    for n in range(4):
        _mark(f'a{n}', nc)
        # filler order ~= data-readiness order, so no queued DMA ever
        # blocks an SP-queue successor that could already run: x loads
        # (no waits) first, then proj/RS of the finished chunk, then the
        # next n-tile's GEMM/V/M, then LN2.
        filler = []
        if n < 3:
            for j4 in range(4):
                filler.append(mk(ph1_lt, 4 * (n + 1) + j4))
        if n > 0:
            for j in range(4):
                filler.append(mk(proj_lt, n - 1, j))
            filler.append(mk(rs_chunk, n - 1))
        if n < 3:
            for m in range(4):
                filler.append(mk(gemm_m, n + 1, m))
            for j4 in range(4):
                filler.append(mk(emit_v_lt, 4 * (n + 1) + j4))
            filler.append(mk(emit_m_rows, n + 1))
        if n > 1:
            filler.append(mk(ln2_chunk, n - 2))
        if no_fill:
            for f in filler:
                f()
            attn_chunk(n, ())
        else:
            attn_chunk(n, filler)
    _mark('tail', nc)
    for j in range(4):
        proj_lt(3, j)
    rs_chunk(3)
    ln2_chunk(2)
    ln2_chunk(3)


def _prep_inputs(x, ln1_g, ln1_b, in_w, out_w, ln2_g, ln2_b, slopes, smear):
    """Slice/transpose per-core views of the weights (host-side marshaling)."""
    x = np.asarray(x, np.float32)
    in_w = np.asarray(in_w, np.float32)
    out_w = np.asarray(out_w, np.float32)
    ln1_g = np.asarray(ln1_g, np.float32)
    ln1_b = np.asarray(ln1_b, np.float32)
    slopes = np.asarray(slopes, np.float32)
    smear = np.asarray(smear, np.float32)
    w_eff = in_w * ln1_g[None, :]
    qkvb = in_w @ ln1_b
    sig = 1.0 / (1.0 + np.exp(-smear))
    bf = ml_dtypes.bfloat16
    in_maps = []
    for c in range(NCORES):
        b, hg = c // 4, c % 4
        f0 = FL * hg
        wq = w_eff[f0:f0 + FL]
        wk = w_eff[DM + f0:DM + f0 + FL]
        wv = w_eff[2 * DM + f0:2 * DM + f0 + FL]
        sl4 = slopes[4 * hg:4 * hg + 4]
        p = np.arange(128, dtype=np.float32)
        kbv = np.arange(NK, dtype=np.float32) * 128.0
        alibi = sl4[:, None, None] * (kbv[None, None, :] + p[None, :, None])
        aliq = np.maximum(sl4, 0.0)[:, None] * np.arange(L, dtype=np.float32)[None, :]
        in_maps.append({
            "xin": np.ascontiguousarray(x[b]),
            "wqk": np.ascontiguousarray(
                np.concatenate([wq, wk], 0).T).astype(bf),
            "wv": np.ascontiguousarray(wv.T).astype(bf),
            "wo": np.ascontiguousarray(out_w[:, f0:f0 + FL].T).astype(bf),
            "bqkr": np.ascontiguousarray(
                np.concatenate([qkvb[f0:f0 + FL],
                                qkvb[DM + f0:DM + f0 + FL]])[None, :]).astype(bf),
            "bvr": np.ascontiguousarray(
                qkvb[2 * DM + f0:2 * DM + f0 + FL][None, :]).astype(bf),
            "srep": np.repeat(sig[4 * hg:4 * hg + 4], 64)[:, None].astype(np.float32),
            "omsrep": np.repeat(1.0 - sig[4 * hg:4 * hg + 4], 64)[:, None].astype(np.float32),
            "alibi": np.ascontiguousarray(alibi.astype(np.float32)),
            "aliq": np.ascontiguousarray(aliq.astype(np.float32)).astype(bf),
            "ln2g": np.asarray(ln2_g, np.float32),
            "ln2b": np.asarray(ln2_b, np.float32),
        })
    return in_maps


def kernel(**inputs):
    if "nc" not in _CACHE:
        _CACHE["nc"] = _build_program()
    nc = _CACHE["nc"]
    in_maps = _prep_inputs(**inputs)
    res = run_bass_kernel_spmd(nc, in_maps, core_ids=list(range(NCORES)))
    out = np.empty((B, L, DM), np.float32)
    for c in range(NCORES):
        b, hg = c // 4, c % 4
        r = res.results[c]["out"]
        for qb in range(NCH):
            g0 = qb * QB + hg * (QB // 4)
            out[b, g0:g0 + QB // 4, :] = r[qb * (QB // 4):(qb + 1) * (QB // 4), :]
    return out


# revision 75
# speedup vs baseline: 1.2137x; 1.0173x over previous
"""Trainium2 Bass kernel for nn_Attention_49709951484392 (causal attention
block: LN1 -> QKV -> key smearing -> causal attention with learned ALiBi ->
out-proj -> LN2), sharded over 8 NeuronCores.

Sharding: core c handles batch c//4 and head-group c%4 (4 of 16 heads).
Out-projection partial sums are ReduceScatter'ed over each batch's 4-core
group; each core then runs LN2 on its 512-row slice of the output.

Attention runs in transposed orientation S^T[k, q] so that:
  - the ALiBi term slope*j (j = key position) is a per-partition bias folded
    into the Exp activation,
  - the per-query shift M_i (softmax overflow guard) is folded into the QK
    matmul by augmenting kT with a constant row (8.0) and qT with a row
    holding -M_i (65-dim contraction),
  - the softmax denominator is produced by the PV matmul via an extra ones
    column appended to V (row 64 of the PV output),
so no transposes of the attention matrix are needed.  M_i is the bound
(|q_i|^2 + max_j|k_j|^2)/16 + relu(slope)*i >= max_j (q_i.k_j/8 + slope*j),
computed with one augmented column-sum matmul per head; kmax is taken over
only the key n-tiles a query chunk can see, so chunk n's attention starts
right after GEMM n-tile n.

The emission is one fused loop over the four 512-column n-tiles:
  LN1+transpose group n -> QK GEMM n -> V GEMM group n -> -M rows chunk n ->
  attention chunk n (out-proj of chunk n-1 interleaved per head) ->
  ReduceScatter chunk n-1 -> LN2 chunk n-2
which keeps the in-order PE stream dense and hides the collectives.

Activations/weights are bf16 on the PE (f32 PSUM accumulation); the act
table dict is reordered so Exp/Ln/Copy/Square all live in one function set
(no LoadActFuncSet churn).
"""
import sys

import numpy as np
import ml_dtypes

sys.path.insert(0, "/opt/trn_rl_repo")

import concourse.bacc as bacc
import concourse.bass as bass
import concourse.mybir as mybir
import concourse.tile as tile
from concourse.bass_utils import run_bass_kernel_spmd
from concourse.masks import make_identity

F32 = mybir.dt.float32
F32R = mybir.dt.float32r
BF16 = mybir.dt.bfloat16
AF = mybir.ActivationFunctionType
ALU = mybir.AluOpType
AX = mybir.AxisListType

HEADS = 16
DH = 64
DM = 1024
B, L = 2, 2048
EPS = 1e-5
NCORES = 8
HG = 4          # heads per core
FL = HG * DH    # local feature width (256)
QB = 512        # query chunk == n-tile width
NK = L // 128   # key blocks of 128
NLT = L // 128  # l-tiles
NCH = L // QB   # chunks (4)

_CACHE = {}
PHASE_MARKS = []


def _mark(name, nc):
    ids = []
    for k in nc.inst_map.keys():
        if isinstance(k, str) and k.startswith("I-"):
            try:
                ids.append(int(k.split("-")[1]))
            except ValueError:
                pass
    PHASE_MARKS.append((name, max(ids) if ids else 0))


def _patch_act_tables():
    """Put the set containing both Exp and Ln first so the act-table pass
    assigns every activation in this kernel to one set (zero reloads)."""
    import concourse.hw_specs as hws
    if getattr(bacc, "_act_tables_patched", False):
        return
    orig = hws.get_activation_tables

    def constrained(module_arch):
        # Keep canonical set order/ids (the runtime keys tables by id), but
        # hide Exp/Ln from every set except the one containing both, so the
        # table-load pass assigns all our activations to that single set.
        t = orig(module_arch)
        import concourse.mybir as mb
        AFt = mb.ActivationFunctionType
        want = {AFt.Exp, AFt.Ln, AFt.Copy, AFt.Square}
        best = None
        for name, funcs in t.items():
            if want <= funcs:
                best = name
                break
        if best is None:
            return t
        out = {}
        for name, funcs in t.items():
            if name == best:
                out[name] = funcs
            else:
                out[name] = funcs - {AFt.Exp, AFt.Ln}
        return out

    bacc.get_activation_tables = constrained
    bacc._act_tables_patched = True


def _build_program():
    _patch_act_tables()
    nc = bacc.Bacc()
    xin = nc.declare_dram_parameter("xin", [L, DM], F32, isOutput=False)
    wqk_d = nc.declare_dram_parameter("wqk", [DM, 2 * FL], BF16, isOutput=False)
    wv_d = nc.declare_dram_parameter("wv", [DM, FL], BF16, isOutput=False)
    wo_d = nc.declare_dram_parameter("wo", [FL, DM], BF16, isOutput=False)
    bqkr_d = nc.declare_dram_parameter("bqkr", [1, 2 * FL], BF16, isOutput=False)
    bvr_d = nc.declare_dram_parameter("bvr", [1, FL], BF16, isOutput=False)
    srep_d = nc.declare_dram_parameter("srep", [FL, 1], F32, isOutput=False)
    omsrep_d = nc.declare_dram_parameter("omsrep", [FL, 1], F32, isOutput=False)
    alibi_d = nc.declare_dram_parameter("alibi", [HG, 128, NK], F32, isOutput=False)
    aliq_d = nc.declare_dram_parameter("aliq", [HG, L], BF16, isOutput=False)
    ln2g_d = nc.declare_dram_parameter("ln2g", [DM], F32, isOutput=False)
    ln2b_d = nc.declare_dram_parameter("ln2b", [DM], F32, isOutput=False)
    out_d = nc.declare_dram_parameter("out", [L // 4, DM], F32, isOutput=True)

    from contextlib import ExitStack
    with tile.TileContext(nc) as tc, ExitStack() as ctx:
        _emit(ctx, nc, tc, xin, wqk_d, wv_d, wo_d, bqkr_d, bvr_d,
              srep_d, omsrep_d, alibi_d, aliq_d, ln2g_d, ln2b_d, out_d)
    nc.compile()
    return nc


def _bcast_ap(handle, parts, free):
    ap = handle[:]
    return bass.AP(tensor=ap.tensor, offset=0, ap=[[0, parts], [1, free]])


def _emit(ctx, nc, tc, xin, wqk_d, wv_d, wo_d, bqkr_d, bvr_d,
          srep_d, omsrep_d, alibi_d, aliq_d, ln2g_d, ln2b_d, out_d):
    consts = ctx.enter_context(tc.tile_pool(name="consts", bufs=1))
    persist = ctx.enter_context(tc.tile_pool(name="persist", bufs=1))
    dram = ctx.enter_context(tc.tile_pool(name="dram", bufs=1, space="DRAM"))

    ident = consts.tile([128, 128], F32)
    make_identity(nc, ident)
    identb = consts.tile([128, 128], BF16)
    nc.vector.tensor_copy(out=identb, in_=ident)
    eps_t = consts.tile([128, 1], F32)
    nc.vector.memset(eps_t, EPS)
    ones64_f = consts.tile([1, 64], F32)
    nc.vector.memset(ones64_f, 1.0)
    ones64_r = consts.tile([1, 64], F32R)
    nc.vector.tensor_copy(out=ones64_r, in_=ones64_f)
    onesvcol_b = consts.tile([128, HG], BF16)
    nc.vector.memset(onesvcol_b, 1.0)
    bd_f = consts.tile([128, 2], F32)
    nc.vector.memset(bd_f, 0.0)
    nc.vector.memset(bd_f[0:64, 0:1], 1.0)
    nc.vector.memset(bd_f[64:128, 1:2], 1.0)
    bd_r = consts.tile([128, 2], F32R)
    nc.vector.tensor_copy(out=bd_r, in_=bd_f)
    # bias matmuls ride in the same PSUM accumulation group as the bf16
    # GEMM chains, so their operands must be bf16 as well
    ones512_b = consts.tile([1, 512], BF16)
    nc.vector.memset(ones512_b, 1.0)
    ones128_b = consts.tile([1, 128], BF16)
    nc.vector.memset(ones128_b, 1.0)
    bqkr_t = consts.tile([1, 2 * FL], BF16)
    nc.scalar.dma_start(out=bqkr_t, in_=bqkr_d[:, :])
    bvr_t = consts.tile([1, FL], BF16)
    nc.scalar.dma_start(out=bvr_t, in_=bvr_d[:, :])

    oms_t = [consts.tile([128, 1], F32, name=f"oms{m}") for m in range(2)]
    for m in range(2):
        nc.scalar.dma_start(out=oms_t[m], in_=omsrep_d[m * 128:(m + 1) * 128, :])
    s_t = [consts.tile([128, 1], F32, name=f"sr{m}") for m in range(2)]
    for m in range(2):
        nc.scalar.dma_start(out=s_t[m], in_=srep_d[m * 128:(m + 1) * 128, :])
    alibi_t = [consts.tile([128, NK], F32, name=f"ali{h}") for h in range(HG)]
    for h in range(HG):
        nc.scalar.dma_start(out=alibi_t[h], in_=alibi_d[h, :, :])

    # persistent activation tiles (qT/kT rows 0:64 = head data, row 64 = aug)
    qT = [persist.tile([65, L], BF16, name=f"qT{h}") for h in range(HG)]
    kT = [persist.tile([65, L], BF16, name=f"kT{h}") for h in range(HG)]

    hTp = ctx.enter_context(tc.tile_pool(name="hTp", bufs=1))
    hT = [hTp.tile([128, 4, L], BF16, name=f"hT{g}") for g in range(2)]
    wp = ctx.enter_context(tc.tile_pool(name="wp", bufs=1))
    wqk8 = wp.tile([128, 8, 2 * FL], BF16, name="wqk8")
    wqk_t = [wqk8[:, kc, :] for kc in range(8)]
    wv8 = wp.tile([128, 8, FL], BF16, name="wv8")
    wv_t = [wv8[:, kc, :] for kc in range(8)]
    wo2 = wp.tile([128, 2, DM], BF16, name="wo2")
    wo_t = [wo2[:, kc, :] for kc in range(2)]

    sqp = ctx.enter_context(tc.tile_pool(name="sqp", bufs=1))
    qn_bf = [sqp.tile([2, L], BF16, name=f"qn{p}") for p in range(2)]
    kmx = [sqp.tile([2, 4], F32, name=f"kmx{p}") for p in range(2)]

    vp = ctx.enter_context(tc.tile_pool(name="vp", bufs=1))
    v_sb = vp.tile([128, NLT, HG, 65], BF16)

    oTp = ctx.enter_context(tc.tile_pool(name="oTp", bufs=1))
    oT = [oTp.tile([128, L], BF16, name=f"oT{m}") for m in range(2)]

    wop = ctx.enter_context(tc.tile_pool(name="wop", bufs=1))
    g2b_t = wop.tile([128, DM], F32)
    nc.scalar.dma_start(out=g2b_t, in_=_bcast_ap(ln2g_d, 128, DM))
    b2b_t = wop.tile([128, DM], F32)
    nc.scalar.dma_start(out=b2b_t, in_=_bcast_ap(ln2b_d, 128, DM))
    g2b_bf = wop.tile([128, DM], BF16)
    nc.vector.tensor_copy(out=g2b_bf, in_=g2b_t)
    b2b_bf = wop.tile([128, DM], BF16)
    nc.vector.tensor_copy(out=b2b_bf, in_=b2b_t)

    ypart = [dram.tile([QB, DM], BF16, name=f"ypart{i}") for i in range(NCH)]
    yred = [dram.tile([QB // 4, DM], BF16, name=f"yred{i}") for i in range(NCH)]

    # working pools
    xp = ctx.enter_context(tc.tile_pool(name="xp", bufs=2))
    x4p = ctx.enter_context(tc.tile_pool(name="x4p", bufs=4))
    stp = ctx.enter_context(tc.tile_pool(name="stp", bufs=6))
    ktp = ctx.enter_context(tc.tile_pool(name="ktp", bufs=1))
    mtp = ctx.enter_context(tc.tile_pool(name="mtp", bufs=2))
    atp = ctx.enter_context(tc.tile_pool(name="atp", bufs=6))
    nrm = ctx.enter_context(tc.tile_pool(name="nrm", bufs=2))
    ysp = ctx.enter_context(tc.tile_pool(name="ysp", bufs=2))

    psW = ctx.enter_context(tc.tile_pool(name="psW", bufs=4, space="PSUM"))
    psO = ctx.enter_context(tc.tile_pool(name="psO", bufs=2, space="PSUM"))
    psv = ctx.enter_context(tc.tile_pool(name="psv", bufs=2, space="PSUM"))

    xr = xin.rearrange("(i j p) d -> i p j d", j=2, p=128)
    state = {"x4": None}
    kbcol = {}

    def ph1_lt(lt):
        # LN1 + PE transpose for one l-tile; x for a pair of l-tiles is
        # DMA'd at its head; weight loads are queued right after the first x.
        if True:
            if lt % 2 == 0:
                x4 = x4p.tile([128, 2, DM], F32, name="x4", tag="x4")
                nc.sync.dma_start(out=x4, in_=xr[lt // 2])
                state["x4"] = x4
                if lt == 0:
                    nc.sync.dma_start(
                        out=wqk8,
                        in_=wqk_d.rearrange("(c p) n -> p c n", p=128))
                    nc.sync.dma_start(
                        out=wv8,
                        in_=wv_d.rearrange("(c p) n -> p c n", p=128))
                    nc.sync.dma_start(
                        out=wo2,
                        in_=wo_d.rearrange("(c p) n -> p c n", p=128))
            x_t = state["x4"][:, lt % 2, :]
            st = stp.tile([128, 2, 6], F32)
            nc.vector.bn_stats(out=st[:, 0, :], in_=x_t[:, 0:512])
            nc.vector.bn_stats(out=st[:, 1, :], in_=x_t[:, 512:1024])
            mv = stp.tile([128, 2], F32)
            nc.vector.bn_aggr(out=mv, in_=st)
            # rstd = exp(-0.5*ln(var+eps)): stays in the Exp/Ln act set
            rstd = stp.tile([128, 1], F32)
            nc.scalar.activation(out=rstd, in_=mv[:, 1:2], func=AF.Ln,
                                 bias=eps_t, scale=1.0)
            nc.scalar.activation(out=rstd, in_=rstd, func=AF.Exp,
                                 bias=0.0, scale=-0.5)
            # LN apply on Pool (SBUF-only op — Pool cannot touch PSUM)
            h_t = xp.tile([128, DM], F32)
            nc.gpsimd.tensor_scalar(out=h_t, in0=x_t, scalar1=mv[:, 0:1],
                                    scalar2=rstd, op0=ALU.subtract,
                                    op1=ALU.mult)
            for g in range(2):
                pst = psW.tile([128, 512], F32, name="pst", tag="w")
                for j in range(4):
                    dc = 4 * g + j
                    nc.tensor.transpose(pst[:, j * 128:(j + 1) * 128],
                                        h_t[:, dc * 128:(dc + 1) * 128],
                                        ident)
                ceng = nc.scalar.copy if g == 0 else \
                    (lambda out, in_: nc.vector.tensor_copy(out=out, in_=in_))
                ceng(out=hT[g][:, :, lt * 128:(lt + 1) * 128],
                     in_=pst.rearrange("p (a b) -> p a b", a=4))

    def gemm_m(n, m):
        # QK GEMM for columns n*512..(n+1)*512, one head-pair m (0,1: q;
        # 2,3: k), plus the inline row-norm stats (for -M) and key smear.
        nsl = slice(n * 512, (n + 1) * 512)
        if True:
            pair = m % 2
            is_q = m < 2
            ps = psW.tile([128, 512], F32, name="psqk", tag="w")
            for kc in range(8):
                nc.tensor.matmul(
                    ps, wqk_t[kc][:, m * 128:(m + 1) * 128],
                    hT[kc // 4][:, kc % 4, nsl],
                    start=(kc == 0), stop=False)
            nc.tensor.matmul(ps, bqkr_t[:, m * 128:(m + 1) * 128],
                             ones512_b, start=False, stop=True)
            # row-norm statistics: sq = (x + b)^2 on ACT, then a blockdiag
            # column-sum -> per-head-pair norms
            sq_t = sqp.tile([128, 512], F32R, name="sq", tag="sq", bufs=2)
            nc.scalar.activation(out=sq_t, in_=ps, func=AF.Square,
                                 bias=0.0, scale=1.0)
            pw2 = psW.tile([128, 512], F32, name="pn2", tag="w")
            pn2 = pw2[0:2, :]
            nc.tensor.matmul(pn2, bd_r, sq_t, start=True, stop=True)
            if is_q:
                nc.scalar.copy(out=qn_bf[pair][:, nsl], in_=pn2)
            else:
                nc.vector.reduce_max(out=kmx[pair][:, n:n + 1],
                                     in_=pn2, axis=AX.X)
            for hh in range(2):
                h = pair * 2 + hh
                rows = slice(hh * 64, (hh + 1) * 64)
                if is_q:
                    nc.scalar.copy(out=qT[h][0:64, nsl], in_=ps[rows, :])
                    continue
                # k already biased: kT = k*(1-s); tmp = k*s; the shifted
                # add completes the smear per column block.  PSUM reads must
                # be DVE; the SBUF-only adds alternate DVE/Pool.
                veng = nc.vector
                aeng = nc.vector if hh == 1 else nc.gpsimd
                veng.tensor_scalar(
                    out=kT[h][0:64, nsl], in0=ps[rows, :],
                    scalar1=oms_t[pair][rows, :], scalar2=None,
                    op0=ALU.mult)
                tmp = ktp.tile([64, 512], BF16, name="ktmp", tag="ktmp",
                               bufs=3)
                veng.tensor_scalar(
                    out=tmp, in0=ps[rows, :],
                    scalar1=s_t[pair][rows, :], scalar2=None,
                    op0=ALU.mult)
                c0 = n * 512
                aeng.tensor_tensor(
                    out=kT[h][0:64, c0 + 1:c0 + 512],
                    in0=kT[h][0:64, c0 + 1:c0 + 512],
                    in1=tmp[:, 0:511], op=ALU.add)
                if n > 0:
                    aeng.tensor_tensor(
                        out=kT[h][0:64, c0:c0 + 1],
                        in0=kT[h][0:64, c0:c0 + 1],
                        in1=kbcol[h][:, 0:1], op=ALU.add)
                if n < 3:
                    bc = ktp.tile([64, 1], BF16, name=f"kb{h}",
                                  tag=f"kb{h}", bufs=2)
                    nc.gpsimd.tensor_copy(out=bc, in_=tmp[:, 511:512])
                    kbcol[h] = bc

    def emit_v_lt(lt):
        if True:
            ps = psv.tile([128, FL], F32, name="psv", tag="psv")
            for kc in range(8):
                nc.tensor.matmul(
                    ps, hT[kc // 4][:, kc % 4, lt * 128:(lt + 1) * 128],
                    wv_t[kc], start=(kc == 0), stop=False)
            nc.tensor.matmul(ps, ones128_b, bvr_t, start=False, stop=True)
            nc.scalar.copy(
                out=v_sb[:, lt, :, 0:64],
                in_=ps.rearrange("p (a b) -> p a b", a=HG))
            nc.vector.tensor_copy(
                out=v_sb[:, lt, :, 64:65],
                in_=onesvcol_b.rearrange("p (a b) -> p a b", a=HG))

    def emit_m_rows(qb):
        # -M = -(qn + kmax^2)/16 - relu(slope)*i over this chunk's queries;
        # kmax over n-tiles 0..qb only (all keys this chunk can see).
        qsl = slice(qb * QB, (qb + 1) * QB)
        for pair in range(2):
            aliq_c = mtp.tile([2, QB], BF16, name="aliqc", tag="aliqc")
            nc.scalar.dma_start(out=aliq_c,
                                in_=aliq_d[pair * 2:pair * 2 + 2, qsl])
            kms2 = mtp.tile([2, 1], F32, name="kms2", tag="kms2")
            nc.vector.reduce_max(out=kms2, in_=kmx[pair][:, 0:qb + 1],
                                 axis=AX.X)
            stag = mtp.tile([2, QB], BF16, name="stag", tag="stag")
            with nc.allow_low_precision(reason="-M guard tolerates bf16"):
                nc.vector.tensor_scalar(out=stag, in0=qn_bf[pair][:, qsl],
                                        scalar1=kms2, scalar2=-1.0 / 16.0,
                                        op0=ALU.add, op1=ALU.mult)
            nc.gpsimd.tensor_tensor(out=stag, in0=stag, in1=aliq_c,
                                    op=ALU.subtract)
            # row 0 is partition-aligned (engine copy); row 1 is not, so it
            # moves via a small SBUF-to-SBUF DMA on the Pool queue
            nc.vector.tensor_copy(out=qT[pair * 2][64:65, qsl],
                                  in_=stag[0:1, :])
            nc.sync.dma_start(out=qT[pair * 2 + 1][64:65, qsl],
                              in_=stag[1:2, :])

    def attn_chunk(qb, filler=()):
        # one software-pipelined stream over (head, k-block): QK+Exp run 3
        # items ahead of PV, crossing head boundaries, so neither PE nor ACT
        # ever drains; each head's normalize is emitted right after its
        # last PV.  `filler` closures (next iteration's transposes/GEMM/V,
        # previous chunk's proj/RS/LN2) are spread through the stream to
        # keep the PE dense and hot.
        qlo = qb * QB
        nkb = (qlo + QB) // 128
        ops_t = {}

        def emit_qk(h, kbi):
            kb = kbi * 128
            off = max(0, kb - qlo)
            sps = psW.tile([128, QB], F32, name="sps", tag="w")
            nc.tensor.matmul(sps[:, off:QB], kT[h][:, kb:kb + 128],
                             qT[h][:, qlo + off:qlo + QB],
                             start=True, stop=True)
            at = atp.tile([128, QB], BF16, name="at", tag="at")
            nc.scalar.activation(out=at[:, off:QB], in_=sps[:, off:QB],
                                 func=AF.Exp,
                                 bias=alibi_t[h][:, kbi:kbi + 1],
                                 scale=0.125)
            if kb >= qlo:
                nc.gpsimd.affine_select(
                    out=at[:, off:off + 128], in_=at[:, off:off + 128],
                    compare_op=ALU.is_ge, fill=0.0, base=0,
                    channel_multiplier=-1, pattern=[[1, 128]])
            return h, kbi, off, at

        def emit_pv(item):
            h, kbi, off, at = item
            if kbi == 0:
                ops_t[h] = psO.tile([65, QB], F32, name="ops", tag="ops")
            nc.tensor.matmul(ops_t[h][:, off:QB], v_sb[:, kbi, h, :],
                             at[:, off:QB],
                             start=(kbi == 0), stop=(kbi == nkb - 1))
            if kbi == nkb - 1:
                emit_norm(h)

        def emit_norm(h):
            ops = ops_t[h]
            dr_r = nrm.tile([1, QB], F32R, name="drr", tag="drr")
            with nc.allow_low_precision(reason="f32r is f32 bits"):
                nc.vector.reciprocal(out=dr_r, in_=ops[64:65, :])
            bps = psW.tile([128, QB], F32, name="bps", tag="w")
            nc.tensor.matmul(bps[0:64, :], ones64_r, dr_r,
                             start=True, stop=True)
            bsb = nrm.tile([64, QB], F32, name="bsb", tag="bsb")
            nc.vector.tensor_copy(out=bsb, in_=bps[0:64, :])
            r0 = (h % 2) * 64
            nc.vector.tensor_mul(out=oT[h // 2][r0:r0 + 64, qlo:qlo + QB],
                                 in0=ops[0:64, :], in1=bsb)

        filler = list(filler)
        n_items = HG * nkb
        done_f = 0
        pend = []
        idx = 0
        for h in range(HG):
            for kbi in range(nkb):
                pend.append(emit_qk(h, kbi))
                if len(pend) > 5:
                    emit_pv(pend.pop(0))
                idx += 1
                want = (idx * len(filler)) // n_items
                while done_f < want:
                    filler[done_f]()
                    done_f += 1
        for item in pend:
            emit_pv(item)
        while done_f < len(filler):
            filler[done_f]()
            done_f += 1

    def proj_lt(qb, j):
        lt = qb * (QB // 128) + j
        ysb = ysp.tile([128, DM], BF16, name="ysb", tag="ysb")
        for n2 in range(2):
            ps = psW.tile([128, 512], F32, name="psy", tag="w")
            for kc in range(2):
                nc.tensor.matmul(ps, oT[kc][:, lt * 128:(lt + 1) * 128],
                                 wo_t[kc][:, n2 * 512:(n2 + 1) * 512],
                                 start=(kc == 0), stop=(kc == 1))
            nc.vector.tensor_copy(out=ysb[:, n2 * 512:(n2 + 1) * 512],
                                  in_=ps)
        nc.sync.dma_start(out=ypart[qb][j * 128:(j + 1) * 128, :], in_=ysb)

    def rs_chunk(qb):
        nc.gpsimd.collective_compute(
            "ReduceScatter", ALU.add,
            replica_groups=[[0, 1, 2, 3], [4, 5, 6, 7]],
            ins=[ypart[qb][:, :]], outs=[yred[qb][:, :]])

    def ln2_chunk(qb):
        y_t = ysp.tile([128, DM], BF16, name="y2t", tag="y2t")
        nc.sync.dma_start(out=y_t, in_=yred[qb][:, :])
        st = ysp.tile([128, 2, 6], F32, name="st2", tag="st2")
        nc.vector.bn_stats(out=st[:, 0, :], in_=y_t[:, 0:512])
        nc.vector.bn_stats(out=st[:, 1, :], in_=y_t[:, 512:1024])
        mv = ysp.tile([128, 2], F32, name="mv2", tag="mv2")
        nc.vector.bn_aggr(out=mv, in_=st)
        rstd = ysp.tile([128, 1], F32, name="rstd2", tag="rstd2")
        nc.scalar.activation(out=rstd, in_=mv[:, 1:2], func=AF.Ln,
                             bias=eps_t, scale=1.0)
        nc.scalar.activation(out=rstd, in_=rstd, func=AF.Exp,
                             bias=0.0, scale=-0.5)
        xh = ysp.tile([128, DM], BF16, name="xh", tag="xh")
        nc.vector.tensor_scalar(out=xh, in0=y_t, scalar1=mv[:, 0:1],
                                scalar2=rstd, op0=ALU.subtract, op1=ALU.mult)
        nc.vector.tensor_tensor(out=xh, in0=xh, in1=g2b_bf, op=ALU.mult)
        o_t = ysp.tile([128, DM], F32, name="o2t", tag="o2t", bufs=1)
        nc.vector.tensor_tensor(out=o_t, in0=xh, in1=b2b_bf, op=ALU.add)
        nc.sync.dma_start(out=out_d[qb * 128:(qb + 1) * 128, :], in_=o_t)

    # ---- fused emission: iteration 0 is emitted straight; afterwards each
    # chunk's attention stream carries the NEXT iteration's LN/transpose/
    # GEMM/V and the PREVIOUS chunk's proj/RS/LN2 as interleaved filler so
    # the PE stream stays dense (and hot) end to end. ----
    def mk(f, *a):
        return lambda: f(*a)

    import os
    no_fill = os.environ.get("KNOFILL", "0") == "1"

    _mark('g0', nc)
    for lt in range(4):
        ph1_lt(lt)
    # kT row 64 = 8.0 (augmentation constant); emitted after the first LN
    # applies so the Pool queue isn't clogged at t=0
    for h in range(HG):
        nc.gpsimd.memset(kT[h][64:65, :], 8.0)
    for m in range(4):
        gemm_m(0, m)
    for lt in range(4):
        emit_v_lt(lt)
    emit_m_rows(0)
    for n in range(4):
        _mark(f'a{n}', nc)
        # filler order ~= data-readiness order, so no queued DMA ever
        # blocks an SP-queue successor that could already run: x loads
        # (no waits) first, then proj/RS of the finished chunk, then the
        # next n-tile's GEMM/V/M, then LN2.
        filler = []
        if n < 3:
            for j4 in range(4):
                filler.append(mk(ph1_lt, 4 * (n + 1) + j4))
        if n > 0:
            for j in range(4):
                filler.append(mk(proj_lt, n - 1, j))
            filler.append(mk(rs_chunk, n - 1))
        if n < 3:
            for m in range(4):
                filler.append(mk(gemm_m, n + 1, m))
            for j4 in range(4):
                filler.append(mk(emit_v_lt, 4 * (n + 1) + j4))
            filler.append(mk(emit_m_rows, n + 1))
        if n > 1:
            filler.append(mk(ln2_chunk, n - 2))
        if no_fill:
            for f in filler:
                f()
            attn_chunk(n, ())
        else:
            attn_chunk(n, filler)
    _mark('tail', nc)
    for j in range(4):
        proj_lt(3, j)
    rs_chunk(3)
    ln2_chunk(2)
    ln2_chunk(3)


def _prep_inputs(x, ln1_g, ln1_b, in_w, out_w, ln2_g, ln2_b, slopes, smear):
    """Slice/transpose per-core views of the weights (host-side marshaling)."""
    x = np.asarray(x, np.float32)
    in_w = np.asarray(in_w, np.float32)
    out_w = np.asarray(out_w, np.float32)
    ln1_g = np.asarray(ln1_g, np.float32)
    ln1_b = np.asarray(ln1_b, np.float32)
    slopes = np.asarray(slopes, np.float32)
    smear = np.asarray(smear, np.float32)
    w_eff = in_w * ln1_g[None, :]
    qkvb = in_w @ ln1_b
    sig = 1.0 / (1.0 + np.exp(-smear))
    bf = ml_dtypes.bfloat16
    in_maps = []
    for c in range(NCORES):
        b, hg = c // 4, c % 4
        f0 = FL * hg
        wq = w_eff[f0:f0 + FL]
        wk = w_eff[DM + f0:DM + f0 + FL]
        wv = w_eff[2 * DM + f0:2 * DM + f0 + FL]
        sl4 = slopes[4 * hg:4 * hg + 4]
        p = np.arange(128, dtype=np.float32)
        kbv = np.arange(NK, dtype=np.float32) * 128.0
        alibi = sl4[:, None, None] * (kbv[None, None, :] + p[None, :, None])
        aliq = np.maximum(sl4, 0.0)[:, None] * np.arange(L, dtype=np.float32)[None, :]
        in_maps.append({
            "xin": np.ascontiguousarray(x[b]),
            "wqk": np.ascontiguousarray(
                np.concatenate([wq, wk], 0).T).astype(bf),
            "wv": np.ascontiguousarray(wv.T).astype(bf),
            "wo": np.ascontiguousarray(out_w[:, f0:f0 + FL].T).astype(bf),
            "bqkr": np.ascontiguousarray(
                np.concatenate([qkvb[f0:f0 + FL],
                                qkvb[DM + f0:DM + f0 + FL]])[None, :]).astype(bf),
            "bvr": np.ascontiguousarray(
                qkvb[2 * DM + f0:2 * DM + f0 + FL][None, :]).astype(bf),
            "srep": np.repeat(sig[4 * hg:4 * hg + 4], 64)[:, None].astype(np.float32),
            "omsrep": np.repeat(1.0 - sig[4 * hg:4 * hg + 4], 64)[:, None].astype(np.float32),
            "alibi": np.ascontiguousarray(alibi.astype(np.float32)),
            "aliq": np.ascontiguousarray(aliq.astype(np.float32)).astype(bf),
            "ln2g": np.asarray(ln2_g, np.float32),
            "ln2b": np.asarray(ln2_b, np.float32),
        })
    return in_maps


def kernel(**inputs):
    if "nc" not in _CACHE:
        _CACHE["nc"] = _build_program()
    nc = _CACHE["nc"]
    in_maps = _prep_inputs(**inputs)
    res = run_bass_kernel_spmd(nc, in_maps, core_ids=list(range(NCORES)))
    out = np.empty((B, L, DM), np.float32)
    for c in range(NCORES):
        b, hg = c // 4, c % 4
        r = res.results[c]["out"]
        for qb in range(NCH):
            g0 = qb * QB + hg * (QB // 4)
            out[b, g0:g0 + QB // 4, :] = r[qb * (QB // 4):(qb + 1) * (QB // 4), :]
    return out


# revision 80
# speedup vs baseline: 1.2541x; 1.0333x over previous
"""Trainium2 Bass kernel for nn_Attention_49709951484392 (causal attention
block: LN1 -> QKV -> key smearing -> causal attention with learned ALiBi ->
out-proj -> LN2), sharded over 8 NeuronCores.

Sharding: core c handles batch c//4 and head-group c%4 (4 of 16 heads).
Out-projection partial sums are ReduceScatter'ed over each batch's 4-core
group; each core then runs LN2 on its 512-row slice of the output.

Attention runs in transposed orientation S^T[k, q] so that:
  - the ALiBi term slope*j (j = key position) is a per-partition bias folded
    into the Exp activation,
  - the per-query shift M_i (softmax overflow guard) is folded into the QK
    matmul by augmenting kT with a constant row (8.0) and qT with a row
    holding -M_i (65-dim contraction),
  - the softmax denominator is produced by the PV matmul via an extra ones
    column appended to V (row 64 of the PV output),
so no transposes of the attention matrix are needed.  M_i is the bound
(|q_i|^2 + max_j|k_j|^2)/16 + relu(slope)*i >= max_j (q_i.k_j/8 + slope*j),
computed with one augmented column-sum matmul per head; kmax is taken over
only the key n-tiles a query chunk can see, so chunk n's attention starts
right after GEMM n-tile n.

The emission is one fused loop over the four 512-column n-tiles:
  LN1+transpose group n -> QK GEMM n -> V GEMM group n -> -M rows chunk n ->
  attention chunk n (out-proj of chunk n-1 interleaved per head) ->
  ReduceScatter chunk n-1 -> LN2 chunk n-2
which keeps the in-order PE stream dense and hides the collectives.

Activations/weights are bf16 on the PE (f32 PSUM accumulation); the act
table dict is reordered so Exp/Ln/Copy/Square all live in one function set
(no LoadActFuncSet churn).
"""
import sys

import numpy as np
import ml_dtypes

sys.path.insert(0, "/opt/trn_rl_repo")

import concourse.bacc as bacc
import concourse.bass as bass
import concourse.mybir as mybir
import concourse.tile as tile
from concourse.bass_utils import run_bass_kernel_spmd
from concourse.masks import make_identity

F32 = mybir.dt.float32
F32R = mybir.dt.float32r
BF16 = mybir.dt.bfloat16
AF = mybir.ActivationFunctionType
ALU = mybir.AluOpType
AX = mybir.AxisListType

HEADS = 16
DH = 64
DM = 1024
B, L = 2, 2048
EPS = 1e-5
NCORES = 8
HG = 4          # heads per core
FL = HG * DH    # local feature width (256)
QB = 512        # query chunk == n-tile width
NK = L // 128   # key blocks of 128
NLT = L // 128  # l-tiles
NCH = L // QB   # chunks (4)

_CACHE = {}
PHASE_MARKS = []


def _mark(name, nc):
    ids = []
    for k in nc.inst_map.keys():
        if isinstance(k, str) and k.startswith("I-"):
            try:
                ids.append(int(k.split("-")[1]))
            except ValueError:
                pass
    PHASE_MARKS.append((name, max(ids) if ids else 0))


def _patch_act_tables():
    """Put the set containing both Exp and Ln first so the act-table pass
    assigns every activation in this kernel to one set (zero reloads)."""
    import concourse.hw_specs as hws
    if getattr(bacc, "_act_tables_patched", False):
        return
    orig = hws.get_activation_tables

    def constrained(module_arch):
        # Keep canonical set order/ids (the runtime keys tables by id), but
        # hide Exp/Ln from every set except the one containing both, so the
        # table-load pass assigns all our activations to that single set.
        t = orig(module_arch)
        import concourse.mybir as mb
        AFt = mb.ActivationFunctionType
        want = {AFt.Exp, AFt.Ln, AFt.Copy, AFt.Square}
        best = None
        for name, funcs in t.items():
            if want <= funcs:
                best = name
                break
        if best is None:
            return t
        out = {}
        for name, funcs in t.items():
            if name == best:
                out[name] = funcs
            else:
                out[name] = funcs - {AFt.Exp, AFt.Ln}
        return out

    bacc.get_activation_tables = constrained
    bacc._act_tables_patched = True


def _build_program():
    _patch_act_tables()
    nc = bacc.Bacc()
    xin = nc.declare_dram_parameter("xin", [L, DM], F32, isOutput=False)
    wqk_d = nc.declare_dram_parameter("wqk", [DM, 2 * FL], BF16, isOutput=False)
    wv_d = nc.declare_dram_parameter("wv", [DM, FL], BF16, isOutput=False)
    wo_d = nc.declare_dram_parameter("wo", [FL, DM], BF16, isOutput=False)
    bqkr_d = nc.declare_dram_parameter("bqkr", [1, 2 * FL], BF16, isOutput=False)
    bvr_d = nc.declare_dram_parameter("bvr", [1, FL], BF16, isOutput=False)
    srep_d = nc.declare_dram_parameter("srep", [FL, 1], F32, isOutput=False)
    omsrep_d = nc.declare_dram_parameter("omsrep", [FL, 1], F32, isOutput=False)
    alibi_d = nc.declare_dram_parameter("alibi", [HG, 128, NK], F32, isOutput=False)
    aliq_d = nc.declare_dram_parameter("aliq", [HG, L], BF16, isOutput=False)
    ln2g_d = nc.declare_dram_parameter("ln2g", [DM], F32, isOutput=False)
    ln2b_d = nc.declare_dram_parameter("ln2b", [DM], F32, isOutput=False)
    out_d = nc.declare_dram_parameter("out", [L // 4, DM], F32, isOutput=True)

    from contextlib import ExitStack
    with tile.TileContext(nc) as tc, ExitStack() as ctx:
        _emit(ctx, nc, tc, xin, wqk_d, wv_d, wo_d, bqkr_d, bvr_d,
              srep_d, omsrep_d, alibi_d, aliq_d, ln2g_d, ln2b_d, out_d)
    nc.compile()
    return nc


def _bcast_ap(handle, parts, free):
    ap = handle[:]
    return bass.AP(tensor=ap.tensor, offset=0, ap=[[0, parts], [1, free]])


def _emit(ctx, nc, tc, xin, wqk_d, wv_d, wo_d, bqkr_d, bvr_d,
          srep_d, omsrep_d, alibi_d, aliq_d, ln2g_d, ln2b_d, out_d):
    consts = ctx.enter_context(tc.tile_pool(name="consts", bufs=1))
    persist = ctx.enter_context(tc.tile_pool(name="persist", bufs=1))
    dram = ctx.enter_context(tc.tile_pool(name="dram", bufs=1, space="DRAM"))

    ident = consts.tile([128, 128], F32)
    make_identity(nc, ident)
    identb = consts.tile([128, 128], BF16)
    nc.vector.tensor_copy(out=identb, in_=ident)
    eps_t = consts.tile([128, 1], F32)
    nc.vector.memset(eps_t, EPS)
    ones64_f = consts.tile([1, 64], F32)
    nc.vector.memset(ones64_f, 1.0)
    ones64_r = consts.tile([1, 64], F32R)
    nc.vector.tensor_copy(out=ones64_r, in_=ones64_f)
    onesvcol_b = consts.tile([128, HG], BF16)
    nc.vector.memset(onesvcol_b, 1.0)
    bd_f = consts.tile([128, 2], F32)
    nc.vector.memset(bd_f, 0.0)
    nc.vector.memset(bd_f[0:64, 0:1], 1.0)
    nc.vector.memset(bd_f[64:128, 1:2], 1.0)
    bd_r = consts.tile([128, 2], F32R)
    nc.vector.tensor_copy(out=bd_r, in_=bd_f)
    # bias matmuls ride in the same PSUM accumulation group as the bf16
    # GEMM chains, so their operands must be bf16 as well
    ones512_b = consts.tile([1, 512], BF16)
    nc.vector.memset(ones512_b, 1.0)
    ones128_b = consts.tile([1, 128], BF16)
    nc.vector.memset(ones128_b, 1.0)
    bqkr_t = consts.tile([1, 2 * FL], BF16)
    nc.scalar.dma_start(out=bqkr_t, in_=bqkr_d[:, :])
    bvr_t = consts.tile([1, FL], BF16)
    nc.scalar.dma_start(out=bvr_t, in_=bvr_d[:, :])

    oms_t = [consts.tile([128, 1], F32, name=f"oms{m}") for m in range(2)]
    for m in range(2):
        nc.scalar.dma_start(out=oms_t[m], in_=omsrep_d[m * 128:(m + 1) * 128, :])
    s_t = [consts.tile([128, 1], F32, name=f"sr{m}") for m in range(2)]
    for m in range(2):
        nc.scalar.dma_start(out=s_t[m], in_=srep_d[m * 128:(m + 1) * 128, :])
    alibi_t = [consts.tile([128, NK], F32, name=f"ali{h}") for h in range(HG)]
    for h in range(HG):
        nc.scalar.dma_start(out=alibi_t[h], in_=alibi_d[h, :, :])

    # persistent activation tiles (qT/kT rows 0:64 = head data, row 64 = aug)
    qT = [persist.tile([65, L], BF16, name=f"qT{h}") for h in range(HG)]
    kT = [persist.tile([65, L], BF16, name=f"kT{h}") for h in range(HG)]

    hTp = ctx.enter_context(tc.tile_pool(name="hTp", bufs=1))
    hT = [hTp.tile([128, 4, L], BF16, name=f"hT{g}") for g in range(2)]
    wp = ctx.enter_context(tc.tile_pool(name="wp", bufs=1))
    wqk8 = wp.tile([128, 8, 2 * FL], BF16, name="wqk8")
    wqk_t = [wqk8[:, kc, :] for kc in range(8)]
    wv8 = wp.tile([128, 8, FL], BF16, name="wv8")
    wv_t = [wv8[:, kc, :] for kc in range(8)]
    wo2 = wp.tile([128, 2, DM], BF16, name="wo2")
    wo_t = [wo2[:, kc, :] for kc in range(2)]

    sqp = ctx.enter_context(tc.tile_pool(name="sqp", bufs=1))
    qn_bf = [sqp.tile([2, L], BF16, name=f"qn{p}") for p in range(2)]
    kmx = [sqp.tile([2, 4], F32, name=f"kmx{p}") for p in range(2)]

    vp = ctx.enter_context(tc.tile_pool(name="vp", bufs=1))
    v_sb = vp.tile([128, NLT, HG, 65], BF16)

    oTp = ctx.enter_context(tc.tile_pool(name="oTp", bufs=1))
    oT = [oTp.tile([128, L], BF16, name=f"oT{m}") for m in range(2)]

    wop = ctx.enter_context(tc.tile_pool(name="wop", bufs=1))
    g2b_t = wop.tile([128, DM], F32)
    nc.scalar.dma_start(out=g2b_t, in_=_bcast_ap(ln2g_d, 128, DM))
    b2b_t = wop.tile([128, DM], F32)
    nc.scalar.dma_start(out=b2b_t, in_=_bcast_ap(ln2b_d, 128, DM))
    g2b_bf = wop.tile([128, DM], BF16)
    nc.vector.tensor_copy(out=g2b_bf, in_=g2b_t)
    b2b_bf = wop.tile([128, DM], BF16)
    nc.vector.tensor_copy(out=b2b_bf, in_=b2b_t)

    ypart = [dram.tile([QB, DM], BF16, name=f"ypart{i}") for i in range(NCH)]
    yred = [dram.tile([QB // 4, DM], BF16, name=f"yred{i}") for i in range(NCH)]

    # working pools
    xp = ctx.enter_context(tc.tile_pool(name="xp", bufs=2))
    x4p = ctx.enter_context(tc.tile_pool(name="x4p", bufs=4))
    stp = ctx.enter_context(tc.tile_pool(name="stp", bufs=8))
    ktp = ctx.enter_context(tc.tile_pool(name="ktp", bufs=1))
    mtp = ctx.enter_context(tc.tile_pool(name="mtp", bufs=2))
    atp = ctx.enter_context(tc.tile_pool(name="atp", bufs=6))
    nrm = ctx.enter_context(tc.tile_pool(name="nrm", bufs=2))
    ysp = ctx.enter_context(tc.tile_pool(name="ysp", bufs=2))

    psW = ctx.enter_context(tc.tile_pool(name="psW", bufs=4, space="PSUM"))
    psO = ctx.enter_context(tc.tile_pool(name="psO", bufs=2, space="PSUM"))
    psv = ctx.enter_context(tc.tile_pool(name="psv", bufs=2, space="PSUM"))

    xr = xin.rearrange("(i j p) d -> i p j d", j=2, p=128)
    state = {"x4": None}
    kbcol = {}

    def ph1_lt(lt):
        # LN1 + PE transpose for one l-tile; x for a pair of l-tiles is
        # DMA'd at its head; weight loads are queued right after the first x.
        if True:
            if lt % 2 == 0:
                x4 = x4p.tile([128, 2, DM], F32, name="x4", tag="x4")
                nc.sync.dma_start(out=x4, in_=xr[lt // 2])
                state["x4"] = x4
                if lt == 0:
                    nc.sync.dma_start(
                        out=wqk8,
                        in_=wqk_d.rearrange("(c p) n -> p c n", p=128))
                    nc.sync.dma_start(
                        out=wv8,
                        in_=wv_d.rearrange("(c p) n -> p c n", p=128))
                    nc.sync.dma_start(
                        out=wo2,
                        in_=wo_d.rearrange("(c p) n -> p c n", p=128))
            x_t = state["x4"][:, lt % 2, :]
            st = stp.tile([128, 2, 6], F32)
            nc.vector.bn_stats(out=st[:, 0, :], in_=x_t[:, 0:512])
            nc.vector.bn_stats(out=st[:, 1, :], in_=x_t[:, 512:1024])
            mv = stp.tile([128, 2], F32)
            nc.vector.bn_aggr(out=mv, in_=st)
            # rstd = exp(-0.5*ln(var+eps)): stays in the Exp/Ln act set
            rstd = stp.tile([128, 1], F32)
            nc.scalar.activation(out=rstd, in_=mv[:, 1:2], func=AF.Ln,
                                 bias=eps_t, scale=1.0)
            nc.scalar.activation(out=rstd, in_=rstd, func=AF.Exp,
                                 bias=0.0, scale=-0.5)
            # LN apply on Pool (SBUF-only op — Pool cannot touch PSUM)
            h_t = xp.tile([128, DM], F32)
            nc.gpsimd.tensor_scalar(out=h_t, in0=x_t, scalar1=mv[:, 0:1],
                                    scalar2=rstd, op0=ALU.subtract,
                                    op1=ALU.mult)
            for g in range(2):
                pst = psW.tile([128, 512], F32, name="pst", tag="w")
                for j in range(4):
                    dc = 4 * g + j
                    nc.tensor.transpose(pst[:, j * 128:(j + 1) * 128],
                                        h_t[:, dc * 128:(dc + 1) * 128],
                                        ident)
                ceng = nc.scalar.copy if g == 0 else \
                    (lambda out, in_: nc.vector.tensor_copy(out=out, in_=in_))
                ceng(out=hT[g][:, :, lt * 128:(lt + 1) * 128],
                     in_=pst.rearrange("p (a b) -> p a b", a=4))

    def gemm_m(n, m):
        # QK GEMM for columns n*512..(n+1)*512, one head-pair m (0,1: q;
        # 2,3: k), plus the inline row-norm stats (for -M) and key smear.
        nsl = slice(n * 512, (n + 1) * 512)
        if True:
            pair = m % 2
            is_q = m < 2
            ps = psW.tile([128, 512], F32, name="psqk", tag="w")
            for kc in range(8):
                nc.tensor.matmul(
                    ps, wqk_t[kc][:, m * 128:(m + 1) * 128],
                    hT[kc // 4][:, kc % 4, nsl],
                    start=(kc == 0), stop=False)
            nc.tensor.matmul(ps, bqkr_t[:, m * 128:(m + 1) * 128],
                             ones512_b, start=False, stop=True)
            # row-norm statistics: sq = (x + b)^2 on ACT, then a blockdiag
            # column-sum -> per-head-pair norms
            sq_t = sqp.tile([128, 512], F32R, name="sq", tag="sq", bufs=2)
            nc.scalar.activation(out=sq_t, in_=ps, func=AF.Square,
                                 bias=0.0, scale=1.0)
            pw2 = psW.tile([128, 512], F32, name="pn2", tag="w")
            pn2 = pw2[0:2, :]
            nc.tensor.matmul(pn2, bd_r, sq_t, start=True, stop=True)
            if is_q:
                nc.scalar.copy(out=qn_bf[pair][:, nsl], in_=pn2)
            else:
                nc.vector.reduce_max(out=kmx[pair][:, n:n + 1],
                                     in_=pn2, axis=AX.X)
            for hh in range(2):
                h = pair * 2 + hh
                rows = slice(hh * 64, (hh + 1) * 64)
                if is_q:
                    nc.scalar.copy(out=qT[h][0:64, nsl], in_=ps[rows, :])
                    continue
                # k already biased: kT = k*(1-s); tmp = k*s; the shifted
                # add completes the smear per column block.  PSUM reads must
                # be DVE; the SBUF-only adds alternate DVE/Pool.
                veng = nc.vector
                aeng = nc.vector if hh == 1 else nc.gpsimd
                veng.tensor_scalar(
                    out=kT[h][0:64, nsl], in0=ps[rows, :],
                    scalar1=oms_t[pair][rows, :], scalar2=None,
                    op0=ALU.mult)
                tmp = ktp.tile([64, 512], BF16, name="ktmp", tag="ktmp",
                               bufs=3)
                veng.tensor_scalar(
                    out=tmp, in0=ps[rows, :],
                    scalar1=s_t[pair][rows, :], scalar2=None,
                    op0=ALU.mult)
                c0 = n * 512
                aeng.tensor_tensor(
                    out=kT[h][0:64, c0 + 1:c0 + 512],
                    in0=kT[h][0:64, c0 + 1:c0 + 512],
                    in1=tmp[:, 0:511], op=ALU.add)
                if n > 0:
                    aeng.tensor_tensor(
                        out=kT[h][0:64, c0:c0 + 1],
                        in0=kT[h][0:64, c0:c0 + 1],
                        in1=kbcol[h][:, 0:1], op=ALU.add)
                if n < 3:
                    bc = ktp.tile([64, 1], BF16, name=f"kb{h}",
                                  tag=f"kb{h}", bufs=2)
                    nc.gpsimd.tensor_copy(out=bc, in_=tmp[:, 511:512])
                    kbcol[h] = bc

    def emit_v_lt(lt):
        if True:
            ps = psv.tile([128, FL], F32, name="psv", tag="psv")
            for kc in range(8):
                nc.tensor.matmul(
                    ps, hT[kc // 4][:, kc % 4, lt * 128:(lt + 1) * 128],
                    wv_t[kc], start=(kc == 0), stop=False)
            nc.tensor.matmul(ps, ones128_b, bvr_t, start=False, stop=True)
            nc.scalar.copy(
                out=v_sb[:, lt, :, 0:64],
                in_=ps.rearrange("p (a b) -> p a b", a=HG))
            nc.vector.tensor_copy(
                out=v_sb[:, lt, :, 64:65],
                in_=onesvcol_b.rearrange("p (a b) -> p a b", a=HG))

    def emit_m_rows(qb):
        # -M = -(qn + kmax^2)/16 - relu(slope)*i over this chunk's queries;
        # kmax over n-tiles 0..qb only (all keys this chunk can see).
        qsl = slice(qb * QB, (qb + 1) * QB)
        for pair in range(2):
            aliq_c = mtp.tile([2, QB], BF16, name="aliqc", tag="aliqc")
            nc.scalar.dma_start(out=aliq_c,
                                in_=aliq_d[pair * 2:pair * 2 + 2, qsl])
            kms2 = mtp.tile([2, 1], F32, name="kms2", tag="kms2")
            nc.vector.reduce_max(out=kms2, in_=kmx[pair][:, 0:qb + 1],
                                 axis=AX.X)
            stag = mtp.tile([2, QB], BF16, name="stag", tag="stag")
            with nc.allow_low_precision(reason="-M guard tolerates bf16"):
                nc.vector.tensor_scalar(out=stag, in0=qn_bf[pair][:, qsl],
                                        scalar1=kms2, scalar2=-1.0 / 16.0,
                                        op0=ALU.add, op1=ALU.mult)
            nc.gpsimd.tensor_tensor(out=stag, in0=stag, in1=aliq_c,
                                    op=ALU.subtract)
            # row 0 is partition-aligned (engine copy); row 1 is not, so it
            # moves via a small SBUF-to-SBUF DMA on the Pool queue
            nc.vector.tensor_copy(out=qT[pair * 2][64:65, qsl],
                                  in_=stag[0:1, :])
            nc.sync.dma_start(out=qT[pair * 2 + 1][64:65, qsl],
                              in_=stag[1:2, :])

    # one software-pipelined stream over (chunk, head, k-block): QK+Exp run
    # 5 items ahead of PV, crossing head AND chunk boundaries, so neither PE
    # nor ACT ever drains; each head's normalize is emitted right after its
    # last PV.  Filler closures are spread through the stream to keep the PE
    # dense and hot.
    ops_t = {}

    def emit_qk(qb, h, kbi):
        qlo = qb * QB
        kb = kbi * 128
        off = max(0, kb - qlo)
        sps = psW.tile([128, QB], F32, name="sps", tag="w")
        nc.tensor.matmul(sps[:, off:QB], kT[h][:, kb:kb + 128],
                         qT[h][:, qlo + off:qlo + QB],
                         start=True, stop=True)
        at = atp.tile([128, QB], BF16, name="at", tag="at")
        nc.scalar.activation(out=at[:, off:QB], in_=sps[:, off:QB],
                             func=AF.Exp,
                             bias=alibi_t[h][:, kbi:kbi + 1],
                             scale=0.125)
        if kb >= qlo:
            nc.gpsimd.affine_select(
                out=at[:, off:off + 128], in_=at[:, off:off + 128],
                compare_op=ALU.is_ge, fill=0.0, base=0,
                channel_multiplier=-1, pattern=[[1, 128]])
        return qb, h, kbi, off, at

    def emit_pv(item):
        qb, h, kbi, off, at = item
        nkb = (qb * QB + QB) // 128
        if kbi == 0:
            ops_t[h] = psO.tile([65, QB], F32, name="ops", tag="ops")
        nc.tensor.matmul(ops_t[h][:, off:QB], v_sb[:, kbi, h, :],
                         at[:, off:QB],
                         start=(kbi == 0), stop=(kbi == nkb - 1))
        if kbi == nkb - 1:
            emit_norm(qb, h)

    def emit_norm(qb, h):
        qlo = qb * QB
        ops = ops_t[h]
        dr_r = nrm.tile([1, QB], F32R, name="drr", tag="drr")
        with nc.allow_low_precision(reason="f32r is f32 bits"):
            nc.vector.reciprocal(out=dr_r, in_=ops[64:65, :])
        bps = psW.tile([128, QB], F32, name="bps", tag="w")
        nc.tensor.matmul(bps[0:64, :], ones64_r, dr_r,
                         start=True, stop=True)
        bsb = nrm.tile([64, QB], F32, name="bsb", tag="bsb")
        nc.vector.tensor_copy(out=bsb, in_=bps[0:64, :])
        r0 = (h % 2) * 64
        nc.vector.tensor_mul(out=oT[h // 2][r0:r0 + 64, qlo:qlo + QB],
                             in0=ops[0:64, :], in1=bsb)

    pend = []

    def attn_chunk(qb, filler=()):
        nkb = (qb * QB + QB) // 128
        filler = list(filler)
        n_items = HG * nkb
        done_f = 0
        idx = 0
        for h in range(HG):
            for kbi in range(nkb):
                pend.append(emit_qk(qb, h, kbi))
                if len(pend) > 5:
                    emit_pv(pend.pop(0))
                idx += 1
                want = (idx * len(filler)) // n_items
                while done_f < want:
                    filler[done_f]()
                    done_f += 1
        while done_f < len(filler):
            filler[done_f]()
            done_f += 1

    def attn_drain():
        for item in pend:
            emit_pv(item)
        del pend[:]

    def proj_lt(qb, j):
        lt = qb * (QB // 128) + j
        ysb = ysp.tile([128, DM], BF16, name="ysb", tag="ysb")
        for n2 in range(2):
            ps = psW.tile([128, 512], F32, name="psy", tag="w")
            for kc in range(2):
                nc.tensor.matmul(ps, oT[kc][:, lt * 128:(lt + 1) * 128],
                                 wo_t[kc][:, n2 * 512:(n2 + 1) * 512],
                                 start=(kc == 0), stop=(kc == 1))
            nc.vector.tensor_copy(out=ysb[:, n2 * 512:(n2 + 1) * 512],
                                  in_=ps)
        nc.sync.dma_start(out=ypart[qb][j * 128:(j + 1) * 128, :], in_=ysb)

    def rs_chunk(qb):
        nc.gpsimd.collective_compute(
            "ReduceScatter", ALU.add,
            replica_groups=[[0, 1, 2, 3], [4, 5, 6, 7]],
            ins=[ypart[qb][:, :]], outs=[yred[qb][:, :]])

    def ln2_chunk(qb):
        y_t = ysp.tile([128, DM], BF16, name="y2t", tag="y2t")
        nc.sync.dma_start(out=y_t, in_=yred[qb][:, :])
        st = ysp.tile([128, 2, 6], F32, name="st2", tag="st2")
        nc.vector.bn_stats(out=st[:, 0, :], in_=y_t[:, 0:512])
        nc.vector.bn_stats(out=st[:, 1, :], in_=y_t[:, 512:1024])
        mv = ysp.tile([128, 2], F32, name="mv2", tag="mv2")
        nc.vector.bn_aggr(out=mv, in_=st)
        rstd = ysp.tile([128, 1], F32, name="rstd2", tag="rstd2")
        nc.scalar.activation(out=rstd, in_=mv[:, 1:2], func=AF.Ln,
                             bias=eps_t, scale=1.0)
        nc.scalar.activation(out=rstd, in_=rstd, func=AF.Exp,
                             bias=0.0, scale=-0.5)
        xh = ysp.tile([128, DM], BF16, name="xh", tag="xh")
        nc.vector.tensor_scalar(out=xh, in0=y_t, scalar1=mv[:, 0:1],
                                scalar2=rstd, op0=ALU.subtract, op1=ALU.mult)
        nc.vector.tensor_tensor(out=xh, in0=xh, in1=g2b_bf, op=ALU.mult)
        o_t = ysp.tile([128, DM], F32, name="o2t", tag="o2t", bufs=1)
        nc.vector.tensor_tensor(out=o_t, in0=xh, in1=b2b_bf, op=ALU.add)
        nc.sync.dma_start(out=out_d[qb * 128:(qb + 1) * 128, :], in_=o_t)

    # ---- fused emission: iteration 0 is emitted straight; afterwards each
    # chunk's attention stream carries the NEXT iteration's LN/transpose/
    # GEMM/V and the PREVIOUS chunk's proj/RS/LN2 as interleaved filler so
    # the PE stream stays dense (and hot) end to end. ----
    def mk(f, *a):
        return lambda: f(*a)

    import os
    no_fill = os.environ.get("KNOFILL", "0") == "1"

    _mark('g0', nc)
    for lt in range(4):
        ph1_lt(lt)
    # kT row 64 = 8.0 (augmentation constant); emitted after the first LN
    # applies so the Pool queue isn't clogged at t=0
    for h in range(HG):
        nc.gpsimd.memset(kT[h][64:65, :], 8.0)
    for m in (2, 3, 0, 1):
        gemm_m(0, m)
    for lt in range(4):
        emit_v_lt(lt)
    emit_m_rows(0)
    for n in range(4):
        _mark(f'a{n}', nc)
        # filler order ~= data-readiness order, so no queued DMA ever
        # blocks an SP-queue successor that could already run: x loads
        # (no waits) first, then proj/RS of the finished chunk, then the
        # next n-tile's GEMM/V/M, then LN2.
        filler = []
        if n < 3:
            for j4 in range(4):
                filler.append(mk(ph1_lt, 4 * (n + 1) + j4))
        if n > 0:
            for j in range(4):
                filler.append(mk(proj_lt, n - 1, j))
            filler.append(mk(rs_chunk, n - 1))
        if n < 3:
            for m in (2, 3, 0, 1):
                filler.append(mk(gemm_m, n + 1, m))
            for j4 in range(4):
                filler.append(mk(emit_v_lt, 4 * (n + 1) + j4))
            filler.append(mk(emit_m_rows, n + 1))
        if n > 1:
            filler.append(mk(ln2_chunk, n - 2))
        if no_fill:
            for f in filler:
                f()
            attn_chunk(n, ())
        else:
            attn_chunk(n, filler)
    attn_drain()
    _mark('tail', nc)
    for j in range(4):
        proj_lt(3, j)
    rs_chunk(3)
    ln2_chunk(2)
    ln2_chunk(3)


def _prep_inputs(x, ln1_g, ln1_b, in_w, out_w, ln2_g, ln2_b, slopes, smear):
    """Slice/transpose per-core views of the weights (host-side marshaling)."""
    x = np.asarray(x, np.float32)
    in_w = np.asarray(in_w, np.float32)
    out_w = np.asarray(out_w, np.float32)
    ln1_g = np.asarray(ln1_g, np.float32)
    ln1_b = np.asarray(ln1_b, np.float32)
    slopes = np.asarray(slopes, np.float32)
    smear = np.asarray(smear, np.float32)
    w_eff = in_w * ln1_g[None, :]
    qkvb = in_w @ ln1_b
    sig = 1.0 / (1.0 + np.exp(-smear))
    bf = ml_dtypes.bfloat16
    in_maps = []
    for c in range(NCORES):
        b, hg = c // 4, c % 4
        f0 = FL * hg
        wq = w_eff[f0:f0 + FL]
        wk = w_eff[DM + f0:DM + f0 + FL]
        wv = w_eff[2 * DM + f0:2 * DM + f0 + FL]
        sl4 = slopes[4 * hg:4 * hg + 4]
        p = np.arange(128, dtype=np.float32)
        kbv = np.arange(NK, dtype=np.float32) * 128.0
        alibi = sl4[:, None, None] * (kbv[None, None, :] + p[None, :, None])
        aliq = np.maximum(sl4, 0.0)[:, None] * np.arange(L, dtype=np.float32)[None, :]
        in_maps.append({
            "xin": np.ascontiguousarray(x[b]),
            "wqk": np.ascontiguousarray(
                np.concatenate([wq, wk], 0).T).astype(bf),
            "wv": np.ascontiguousarray(wv.T).astype(bf),
            "wo": np.ascontiguousarray(out_w[:, f0:f0 + FL].T).astype(bf),
            "bqkr": np.ascontiguousarray(
                np.concatenate([qkvb[f0:f0 + FL],
                                qkvb[DM + f0:DM + f0 + FL]])[None, :]).astype(bf),
            "bvr": np.ascontiguousarray(
                qkvb[2 * DM + f0:2 * DM + f0 + FL][None, :]).astype(bf),
            "srep": np.repeat(sig[4 * hg:4 * hg + 4], 64)[:, None].astype(np.float32),
            "omsrep": np.repeat(1.0 - sig[4 * hg:4 * hg + 4], 64)[:, None].astype(np.float32),
            "alibi": np.ascontiguousarray(alibi.astype(np.float32)),
            "aliq": np.ascontiguousarray(aliq.astype(np.float32)).astype(bf),
            "ln2g": np.asarray(ln2_g, np.float32),
            "ln2b": np.asarray(ln2_b, np.float32),
        })
    return in_maps


def kernel(**inputs):
    if "nc" not in _CACHE:
        _CACHE["nc"] = _build_program()
    nc = _CACHE["nc"]
    in_maps = _prep_inputs(**inputs)
    res = run_bass_kernel_spmd(nc, in_maps, core_ids=list(range(NCORES)))
    out = np.empty((B, L, DM), np.float32)
    for c in range(NCORES):
        b, hg = c // 4, c % 4
        r = res.results[c]["out"]
        for qb in range(NCH):
            g0 = qb * QB + hg * (QB // 4)
            out[b, g0:g0 + QB // 4, :] = r[qb * (QB // 4):(qb + 1) * (QB // 4), :]
    return out
